# revision 1
# baseline (speedup 1.0000x reference)
"""Transformer block (LN -> 12-head causal attention -> residual -> LN -> MLP
-> residual) for B=4, T=2048, C=768 on 8 trn2 NeuronCores.

Sharding: core = (batch, token-half). Each core handles one batch's K/V in
full and produces the final output for half the tokens (even or odd 128-token
blocks, which balances the causal-attention triangle). No collectives; all
per-core structural differences are carried in input *data* (host-gathered
xT_mine, causal-boundary mask tiles) so a single SPMD program runs on all 8
cores.

On-chip layout is feature-major ("transposed", [C, T]): LN statistics are
computed with ones-vector matmuls on the tensor engine, Q^T/K^T land directly
in the layout attention wants, softmax runs shift-exp (constant shift, exact
softmax) with the row-sum fused into the P@V matmul via a ones column
appended to V, and residuals/bias adds ride the PSUM->SBUF copies.
"""

import math
import os
import sys

for _p in ("/opt/trn_rl_repo", "/root/.axon_site/_ro/trn_rl_repo"):
    if os.path.isdir(_p) and _p not in sys.path:
        sys.path.append(_p)

import numpy as np
import ml_dtypes

import concourse.bacc as bacc
import concourse.tile as tile
import concourse.mybir as mybir
from concourse import bass_utils
from concourse.alu_op_type import AluOpType
from concourse.tile_rust import add_dep_helper

BF = mybir.dt.bfloat16
FP = mybir.dt.float32
F32R = mybir.dt.float32r
AF = mybir.ActivationFunctionType

B, T, C, H, HD = 4, 2048, 768, 12, 64
EPS = 1e-5
SHIFT = 40.0  # constant softmax shift: exp(s - SHIFT); exact softmax
NP = C // 128  # 6 feature partition-tiles
NT = T // 128  # 16 token blocks
TM = T // 2    # 1024 tokens owned per core
NB = 16        # token blocks
bf16 = ml_dtypes.bfloat16

_cache = {}


def _build(debug=False):
    nc = bacc.Bacc("TRN2", target_bir_lowering=False, debug=False)
    d_xT = nc.dram_tensor("xT", [C, T], FP, kind="ExternalInput").ap()
    d_xTm = nc.dram_tensor("xTm", [C, TM], FP, kind="ExternalInput").ap()
    d_wqkvp = nc.dram_tensor("wqkvp", [C, 4 * C], BF, kind="ExternalInput").ap()
    d_w1 = nc.dram_tensor("w1p", [C, 4 * C], BF, kind="ExternalInput").ap()
    d_w2 = nc.dram_tensor("w2p", [C, 4 * C], BF, kind="ExternalInput").ap()
    d_bias = nc.dram_tensor("biasp", [C, 9], FP, kind="ExternalInput").ap()
    d_bvrow = nc.dram_tensor("bvrow", [1, C], FP, kind="ExternalInput").ap()
    d_masks = nc.dram_tensor("masks", [256, 128], BF, kind="ExternalInput").ap()
    d_out = nc.dram_tensor("outT", [C, TM], FP, kind="ExternalOutput").ap()
    if debug is True:
        debug = ["h", "hm", "KT", "QT", "V", "attnT", "xmid"]
    debug = debug or []
    dbg = {}
    if "h" in debug:
        dbg["h"] = nc.dram_tensor("dbg_h", [C, T], BF, kind="ExternalOutput").ap()
    if "hm" in debug:
        dbg["hm"] = nc.dram_tensor("dbg_hm", [C, TM], BF, kind="ExternalOutput").ap()
    if "KT" in debug:
        dbg["KT"] = nc.dram_tensor("dbg_KT", [C, T], BF, kind="ExternalOutput").ap()
    if "QT" in debug:
        dbg["QT"] = nc.dram_tensor("dbg_QT", [C, TM], BF, kind="ExternalOutput").ap()
    if "V" in debug:
        dbg["V"] = nc.dram_tensor("dbg_V", [T, H * 65], BF, kind="ExternalOutput").ap()
    if "attnT" in debug:
        dbg["attnT"] = nc.dram_tensor("dbg_attnT", [C, TM], BF, kind="ExternalOutput").ap()
    if "xmid" in debug:
        dbg["xmid"] = nc.dram_tensor("dbg_xmid", [C, TM], FP, kind="ExternalOutput").ap()

    with tile.TileContext(nc) as tc:
        _body(nc, tc, d_xT, d_xTm, d_wqkvp, d_w1, d_w2, d_bias, d_bvrow,
              d_masks, d_out, dbg)
    nc.compile()
    return nc


def _ln_stats_rows(nc, tc, small, sps, x_dram, ncols, ones_bf, eps_c, sq_of, tag):
    """Stream x (feature-major, fp32, [C, ncols]) from DRAM; return bf16
    broadcast tiles a_bc, c_bc ([128, ncols]) with h = x*a + c == LN(x)."""
    ntg = ncols // 512
    stats = [sps.tile([33, 512], FP, tag=f"{tag}st{g}", name=f"{tag}st{g}") for g in range(ntg)]
    for ci in range(NP):
        for g in range(ntg):
            csl = slice(g * 512, (g + 1) * 512)
            xt = sq_of["roll"].tile([128, 512], FP, tag="xr", name="xr")
            nc.sync.dma_start(xt[:], x_dram[ci * 128:(ci + 1) * 128, csl])
            xb = sq_of["roll"].tile([128, 512], BF, tag="xb", name="xb")
            nc.vector.tensor_copy(xb[:], xt[:])
            sq = sq_of["roll"].tile([128, 512], BF, tag="sq", name="sq")
            nc.vector.tensor_mul(sq[:], xb[:], xb[:])
            nc.tensor.matmul(stats[g][0:1, :], ones_bf[:], xb[:],
                             start=(ci == 0), stop=(ci == NP - 1),
                             skip_group_check=True)
            nc.tensor.matmul(stats[g][32:33, :], ones_bf[:], sq[:],
                             start=(ci == 0), stop=(ci == NP - 1),
                             skip_group_check=True)
    a_bc = sq_of["bc"].tile([128, ncols], BF, tag=f"{tag}abc", name=f"{tag}abc")
    c_bc = sq_of["bc"].tile([128, ncols], BF, tag=f"{tag}cbc", name=f"{tag}cbc")
    for g in range(ntg):
        sl = slice(g * 512, (g + 1) * 512)
        mu = small.tile([1, 512], FP, tag="mu", name="mu")
        nc.scalar.mul(mu[:], stats[g][0:1, :], 1.0 / C)
        m2 = small.tile([1, 512], FP, tag="m2", name="m2")
        nc.scalar.mul(m2[:], stats[g][32:33, :], 1.0 / C)
        var = small.tile([1, 512], FP, tag="va", name="va")
        # var = m2 - mu*mu
        nc.vector.tensor_mul(var[:], mu[:], mu[:])
        nc.vector.tensor_sub(var[:], m2[:], var[:])
        std = small.tile([1, 512], FP, tag="sd", name="sd")
        nc.scalar.activation(std[:], var[:], AF.Sqrt, bias=eps_c[0:1, 0:1])
        a5 = small.tile([1, 512], FP, tag="a5", name="a5")
        nc.vector.reciprocal(a5[:], std[:])
        # c = -mu * a
        c5 = small.tile([1, 512], FP, tag="c5", name="c5")
        nc.vector.tensor_mul(c5[:], mu[:], a5[:])
        nc.vector.tensor_scalar_mul(c5[:], c5[:], -1.0)
        a5b = small.tile([1, 512], BF, tag="a5b", name="a5b")
        nc.vector.tensor_copy(a5b[:], a5[:])
        c5b = small.tile([1, 512], BF, tag="c5b", name="c5b")
        nc.vector.tensor_copy(c5b[:], c5[:])
        nc.gpsimd.partition_broadcast(a_bc[:, sl], a5b[:])
        nc.gpsimd.partition_broadcast(c_bc[:, sl], c5b[:])
    return a_bc, c_bc


def _body(nc, tc, d_xT, d_xTm, d_wqkvp, d_w1, d_w2, d_bias, d_bvrow,
          d_masks, d_out, dbg={}):
    from contextlib import ExitStack

    es = ExitStack()
    g_const = es.enter_context(tc.tile_pool(name="const", bufs=1))
    g_xmid = es.enter_context(tc.tile_pool(name="xmid", bufs=1))
    xmid = [g_xmid.tile([128, TM], FP, tag=f"xm{i}", name=f"xm{i}") for i in range(NP)]
    w_es = ExitStack()
    kqv_stack = ExitStack()
    g_w = w_es.enter_context(tc.tile_pool(name="wqkvp", bufs=1))
    g_kqv = kqv_stack.enter_context(tc.tile_pool(name="kqv", bufs=1))

    # ---- constants ----
    ones_bf = g_const.tile([128, 1], BF, tag="ones_bf", name="ones_bf")
    nc.vector.memset(ones_bf[:], 1.0)
    eps_c = g_const.tile([128, 1], FP, tag="eps_c", name="eps_c")
    nc.vector.memset(eps_c[:], EPS)
    shift_c = g_const.tile([128, 1], FP, tag="shift_c", name="shift_c")
    nc.vector.memset(shift_c[:], -SHIFT)
    bias_sb = [g_const.tile([128, 9], FP, tag=f"bias{f}", name=f"bias{f}") for f in range(NP)]
    for f in range(NP):
        nc.sync.dma_start(bias_sb[f][:], d_bias[f * 128:(f + 1) * 128, :])
    mask_a = g_const.tile([128, 128], BF, tag="mask_a", name="mask_a")
    nc.sync.dma_start(mask_a[:], d_masks[0:128, :])
    mask_b = g_const.tile([128, 128], BF, tag="mask_b", name="mask_b")
    nc.sync.dma_start(mask_b[:], d_masks[128:256, :])
    bv_row = g_const.tile([1, C], FP, tag="bv_row", name="bv_row")
    nc.sync.dma_start(bv_row[:], d_bvrow[:])
    bv_rb = g_const.tile([1, C], BF, tag="bv_rb", name="bv_rb")
    nc.vector.tensor_copy(bv_rb[:], bv_row[:])
    bv_bc = g_const.tile([128, C], BF, tag="bv_bc", name="bv_bc")
    nc.gpsimd.partition_broadcast(bv_bc[:], bv_rb[:])

    # ---- weights for attention part ----
    w_sb = [g_w.tile([128, 4 * C], BF, tag=f"w{c}", name=f"w{c}") for c in range(NP)]
    for c in range(NP):
        nc.sync.dma_start(w_sb[c][:], d_wqkvp[c * 128:(c + 1) * 128, :])

    # ---- persistent activation storage ----
    KT = [g_kqv.tile([128, T], BF, tag=f"KT{i}", name=f"KT{i}") for i in range(NP)]
    QT = [g_kqv.tile([128, TM], BF, tag=f"QT{i}", name=f"QT{i}") for i in range(NP)]
    Vsb = [g_kqv.tile([128, H * 65], BF, tag=f"V{t}", name=f"V{t}") for t in range(NT)]

    # ================= LN1 + QKV =================
    ln_es = ExitStack()
    g_roll = ln_es.enter_context(tc.tile_pool(name="lnroll", bufs=2))
    g_bc = ln_es.enter_context(tc.tile_pool(name="lnbc", bufs=1))
    g_small = ln_es.enter_context(tc.tile_pool(name="lnsmall", bufs=1))
    g_h = ln_es.enter_context(tc.tile_pool(name="hpool", bufs=1))
    pools = {"roll": g_roll, "bc": g_bc}
    sps_es = ExitStack()
    sps = sps_es.enter_context(tc.tile_pool(name="statps", bufs=1, space="PSUM"))
    a_bc, c_bc = _ln_stats_rows(nc, tc, g_small, sps, d_xT, T, ones_bf, eps_c,
                                pools, "f")
    am_bc, cm_bc = _ln_stats_rows(nc, tc, g_small, sps, d_xTm, TM, ones_bf, eps_c,
                                  pools, "m")
    sps_es.close()

    h = [g_h.tile([128, T], BF, tag=f"h{c}", name=f"h{c}") for c in range(NP)]
    hm = [g_h.tile([128, TM], BF, tag=f"hm{c}", name=f"hm{c}") for c in range(NP)]
    for c in range(NP):
        for ch in range(4):
            csl = slice(ch * 512, (ch + 1) * 512)
            xt = g_roll.tile([128, 512], FP, tag="xr", name="xr")
            nc.sync.dma_start(xt[:], d_xT[c * 128:(c + 1) * 128, csl])
            tmp = g_roll.tile([128, 512], FP, tag="h_tmp", name="h_tmp")
            nc.vector.tensor_mul(tmp[:], xt[:], a_bc[:, csl])
            nc.vector.tensor_add(h[c][:, csl], tmp[:], c_bc[:, csl])
        for ch in range(2):
            csl = slice(ch * 512, (ch + 1) * 512)
            xtm = g_roll.tile([128, 512], FP, tag="xr", name="xr")
            nc.sync.dma_start(xtm[:], d_xTm[c * 128:(c + 1) * 128, csl])
            tmpm = g_roll.tile([128, 512], FP, tag="h_tmp", name="h_tmp")
            nc.vector.tensor_mul(tmpm[:], xtm[:], am_bc[:, csl])
            nc.vector.tensor_add(hm[c][:, csl], tmpm[:], cm_bc[:, csl])

    qkv_es = ExitStack()
    gps = qkv_es.enter_context(tc.tile_pool(name="gemmps", bufs=3, space="PSUM"))

    # K^T [C, T]: lhsT = wk tile, rhs = h
    for f in range(NP):
        for g in range(T // 512):
            ps = gps.tile([128, 512], FP, tag="ps", name="ps")
            for c in range(NP):
                nc.tensor.matmul(ps[:], w_sb[c][:, C + f * 128:C + (f + 1) * 128],
                                 h[c][:, g * 512:(g + 1) * 512],
                                 start=(c == 0), stop=(c == NP - 1))
            nc.vector.tensor_scalar_add(KT[f][:, g * 512:(g + 1) * 512], ps[:],
                                        bias_sb[f][:, 1:2])
    # Q^T [C, TM] from h_mine
    for f in range(NP):
        for g in range(TM // 512):
            ps = gps.tile([128, 512], FP, tag="ps", name="ps")
            for c in range(NP):
                nc.tensor.matmul(ps[:], w_sb[c][:, f * 128:(f + 1) * 128],
                                 hm[c][:, g * 512:(g + 1) * 512],
                                 start=(c == 0), stop=(c == NP - 1))
            nc.vector.tensor_scalar_add(QT[f][:, g * 512:(g + 1) * 512], ps[:],
                                        bias_sb[f][:, 0:1])
    # V natural [T, C] (+ ones col per head): lhsT = h tile, rhs = wv
    for t in range(NT):
        v3 = Vsb[t][:].rearrange("p (h d) -> p h d", d=65)
        nc.vector.memset(v3[:, :, 64:65], 1.0)
        for fs in range(2):
            n = 512 if fs == 0 else 256
            nh = n // 64
            ps = gps.tile([128, 512], FP, tag="ps", name="ps")
            for c in range(NP):
                nc.tensor.matmul(ps[:, 0:n], h[c][:, t * 128:(t + 1) * 128],
                                 w_sb[c][:, 2 * C + fs * 512:2 * C + fs * 512 + n],
                                 start=(c == 0), stop=(c == NP - 1))
            # scatter into per-head 65-wide slots with bias add
            nc.vector.scalar_tensor_tensor(
                v3[:, fs * 8:fs * 8 + nh, 0:64],
                ps[:, 0:n].rearrange("p (h d) -> p h d", d=64),
                0.0,
                bv_bc[:, fs * 512:fs * 512 + n].rearrange("p (h d) -> p h d", d=64),
                AluOpType.add, AluOpType.add)
    for c in range(NP):
        if "h" in dbg:
            nc.sync.dma_start(dbg["h"][c * 128:(c + 1) * 128, :], h[c][:])
        if "hm" in dbg:
            nc.sync.dma_start(dbg["hm"][c * 128:(c + 1) * 128, :], hm[c][:])
        if "KT" in dbg:
            nc.sync.dma_start(dbg["KT"][c * 128:(c + 1) * 128, :], KT[c][:])
        if "QT" in dbg:
            nc.sync.dma_start(dbg["QT"][c * 128:(c + 1) * 128, :], QT[c][:])
    if "V" in dbg:
        for t in range(NT):
            nc.sync.dma_start(dbg["V"][t * 128:(t + 1) * 128, :], Vsb[t][:])
    ln_es.close()
    qkv_es.close()

    attnT_es = ExitStack()
    g_attnT = attnT_es.enter_context(tc.tile_pool(name="attnT", bufs=1))
    attnT = [g_attnT.tile([128, TM], BF, tag=f"aT{i}", name=f"aT{i}") for i in range(NP)]

    # ================= attention =================
    att_es = ExitStack()
    g_wei = att_es.enter_context(tc.tile_pool(name="wei", bufs=3))
    g_asc = att_es.enter_context(tc.tile_pool(name="ascratch", bufs=2))
    ps_s_pool = att_es.enter_context(tc.tile_pool(name="sps", bufs=2, space="PSUM"))
    ps_a_pool = att_es.enter_context(tc.tile_pool(name="aps", bufs=2, space="PSUM"))

    for hh in range(H):
        ht, hp = hh // 2, (hh % 2) * 64
        for g in range(2):
            smax = 8 + 8 * g
            pa = ps_a_pool.tile([65, 512], FP, tag="pa", name="pa")
            for sb in range(smax):
                jmin = max(0, math.ceil((sb - 1 - 8 * g) / 2))
                c0 = jmin * 128
                ps = ps_s_pool.tile([128, 512], FP, tag="ps", name="ps")
                nc.tensor.matmul(ps[:, c0:512],
                                 KT[ht][hp:hp + 64, sb * 128:(sb + 1) * 128],
                                 QT[ht][hp:hp + 64, g * 512 + c0:(g + 1) * 512],
                                 start=True, stop=True)
                wei = g_wei.tile([128, 512], BF, tag="wei", name="wei")
                nc.scalar.activation(wei[:, c0:512], ps[:, c0:512], AF.Exp,
                                     bias=shift_c[:])
                if (sb - 8 * g) % 2 == 0:
                    ja = (sb - 8 * g) // 2
                    if 0 <= ja < 4:
                        nc.vector.tensor_mul(wei[:, ja * 128:(ja + 1) * 128],
                                             wei[:, ja * 128:(ja + 1) * 128],
                                             mask_a[:])
                else:
                    jb = (sb - 1 - 8 * g) // 2
                    if 0 <= jb < 4:
                        nc.vector.tensor_mul(wei[:, jb * 128:(jb + 1) * 128],
                                             wei[:, jb * 128:(jb + 1) * 128],
                                             mask_b[:])
                nc.tensor.matmul(pa[:, c0:512], Vsb[sb][:, hh * 65:(hh + 1) * 65],
                                 wei[:, c0:512], start=(sb == 0),
                                 stop=(sb == smax - 1), skip_group_check=True)
            recip = g_asc.tile([1, 512], FP, tag="recip", name="recip")
            nc.vector.reciprocal(recip[:], pa[64:65, :])
            rb = g_asc.tile([64, 512], FP, tag="rb", name="rb")
            nc.gpsimd.partition_broadcast(rb[:], recip[:])
            nc.vector.tensor_mul(attnT[ht][hp:hp + 64, g * 512:(g + 1) * 512],
                                 pa[0:64, :], rb[:])
    if "attnT" in dbg:
        for c in range(NP):
            nc.sync.dma_start(dbg["attnT"][c * 128:(c + 1) * 128, :], attnT[c][:])
    att_es.close()

    # ================= proj + residual =================
    proj_insts = []
    proj_es = ExitStack()
    gps = proj_es.enter_context(tc.tile_pool(name="pps", bufs=3, space="PSUM"))
    g_pr = proj_es.enter_context(tc.tile_pool(name="projroll", bufs=2))
    for f in range(NP):
        for g in range(TM // 512):
            ps = gps.tile([128, 512], FP, tag="ps", name="ps")
            for c in range(NP):
                nc.tensor.matmul(ps[:], w_sb[c][:, 3 * C + f * 128:3 * C + (f + 1) * 128],
                                 attnT[c][:, g * 512:(g + 1) * 512],
                                 start=(c == 0), stop=(c == NP - 1))
            xr = g_pr.tile([128, 512], FP, tag="xr", name="xr")
            nc.sync.dma_start(xr[:], d_xTm[f * 128:(f + 1) * 128,
                                           g * 512:(g + 1) * 512])
            pi = nc.vector.scalar_tensor_tensor(
                xmid[f][:, g * 512:(g + 1) * 512], ps[:], bias_sb[f][:, 3:4],
                xr[:], AluOpType.add, AluOpType.add)
            proj_insts.append(pi)
    if "xmid" in dbg:
        for c in range(NP):
            nc.sync.dma_start(dbg["xmid"][c * 128:(c + 1) * 128, :], xmid[c][:])
    proj_es.close()
    attnT_es.close()
    kqv_stack.close()
    w_es.close()

    # ================= LN2 + MLP =================
    mlp_es = ExitStack()
    g_w1 = mlp_es.enter_context(tc.tile_pool(name="w1p", bufs=1))
    g_w2 = mlp_es.enter_context(tc.tile_pool(name="w2p", bufs=1))
    g_h2 = mlp_es.enter_context(tc.tile_pool(name="h2p", bufs=1))
    g_r = mlp_es.enter_context(tc.tile_pool(name="rp", bufs=2))
    g_roll2 = mlp_es.enter_context(tc.tile_pool(name="mlproll", bufs=2))
    g_bc2 = mlp_es.enter_context(tc.tile_pool(name="mlpbc", bufs=1))
    g_small2 = mlp_es.enter_context(tc.tile_pool(name="mlpsmall", bufs=1))
    sps2 = mlp_es.enter_context(tc.tile_pool(name="statps2", bufs=1, space="PSUM"))
    gps2 = mlp_es.enter_context(tc.tile_pool(name="mps", bufs=3, space="PSUM"))

    w1_sb = [g_w1.tile([128, 4 * C], BF, tag=f"w1_{c}", name=f"w1_{c}") for c in range(NP)]
    w2_sb = [g_w2.tile([128, 4 * C], BF, tag=f"w2_{c}", name=f"w2_{c}") for c in range(NP)]
    for c in range(NP):
        i1 = nc.sync.dma_start(w1_sb[c][:], d_w1[c * 128:(c + 1) * 128, :])
        i2 = nc.sync.dma_start(w2_sb[c][:], d_w2[c * 128:(c + 1) * 128, :])
        for pi in proj_insts:
            add_dep_helper(i1.ins, pi.ins, reason="mlp w-load after proj")
            add_dep_helper(i2.ins, pi.ins, reason="mlp w-load after proj")

    # LN2 stats from xmid (already in SBUF)
    ntg = TM // 512
    stats = [sps2.tile([33, 512], FP, tag=f"st2{g}", name=f"st2{g}") for g in range(ntg)]
    for ci in range(NP):
        xb = g_roll2.tile([128, TM], BF, tag="xb2", name="xb2")
        nc.vector.tensor_copy(xb[:], xmid[ci][:])
        sq = g_roll2.tile([128, TM], BF, tag="sq2", name="sq2")
        nc.vector.tensor_mul(sq[:], xb[:], xb[:])
        for g in range(ntg):
            nc.tensor.matmul(stats[g][0:1, :], ones_bf[:],
                             xb[:, g * 512:(g + 1) * 512],
                             start=(ci == 0), stop=(ci == NP - 1),
                             skip_group_check=True)
            nc.tensor.matmul(stats[g][32:33, :], ones_bf[:],
                             sq[:, g * 512:(g + 1) * 512],
                             start=(ci == 0), stop=(ci == NP - 1),
                             skip_group_check=True)
    a_bc2 = g_bc2.tile([128, TM], BF, tag="a2bc", name="a2bc")
    c_bc2 = g_bc2.tile([128, TM], BF, tag="c2bc", name="c2bc")
    for g in range(ntg):
        sl = slice(g * 512, (g + 1) * 512)
        mu = g_small2.tile([1, 512], FP, tag="mu2", name="mu2")
        nc.scalar.mul(mu[:], stats[g][0:1, :], 1.0 / C)
        m2 = g_small2.tile([1, 512], FP, tag="m22", name="m22")
        nc.scalar.mul(m2[:], stats[g][32:33, :], 1.0 / C)
        var = g_small2.tile([1, 512], FP, tag="va2", name="va2")
        nc.vector.tensor_mul(var[:], mu[:], mu[:])
        nc.vector.tensor_sub(var[:], m2[:], var[:])
        std = g_small2.tile([1, 512], FP, tag="sd2", name="sd2")
        nc.scalar.activation(std[:], var[:], AF.Sqrt, bias=eps_c[0:1, 0:1])
        a5 = g_small2.tile([1, 512], FP, tag="a52", name="a52")
        nc.vector.reciprocal(a5[:], std[:])
        c5 = g_small2.tile([1, 512], FP, tag="c52", name="c52")
        nc.vector.tensor_mul(c5[:], mu[:], a5[:])
        nc.vector.tensor_scalar_mul(c5[:], c5[:], -1.0)
        a5b = g_small2.tile([1, 512], BF, tag="a5b2", name="a5b2")
        nc.vector.tensor_copy(a5b[:], a5[:])
        c5b = g_small2.tile([1, 512], BF, tag="c5b2", name="c5b2")
        nc.vector.tensor_copy(c5b[:], c5[:])
        nc.gpsimd.partition_broadcast(a_bc2[:, sl], a5b[:])
        nc.gpsimd.partition_broadcast(c_bc2[:, sl], c5b[:])

    h2 = [g_h2.tile([128, TM], BF, tag=f"h2_{c}", name=f"h2_{c}") for c in range(NP)]
    for c in range(NP):
        tmp = g_roll2.tile([128, TM], FP, tag="h2tmp", name="h2tmp")
        nc.vector.tensor_mul(tmp[:], xmid[c][:], a_bc2[:])
        nc.vector.tensor_add(h2[c][:], tmp[:], c_bc2[:])

    for g in range(TM // 512):
        r_tiles = []
        for m in range(24):
            ps = gps2.tile([128, 512], FP, tag="ps", name="ps")
            for c in range(NP):
                nc.tensor.matmul(ps[:], w1_sb[c][:, m * 128:(m + 1) * 128],
                                 h2[c][:, g * 512:(g + 1) * 512],
                                 start=(c == 0), stop=(c == NP - 1))
            r = g_r.tile([128, 512], BF, tag=f"r{m}", name=f"r{m}")
            nc.vector.tensor_scalar(r[:], ps[:], bias_sb[m % 6][:, 5 + m // 6:6 + m // 6],
                                    0.0, AluOpType.add, AluOpType.max)
            r_tiles.append(r)
        for f in range(NP):
            ps = gps2.tile([128, 512], FP, tag="ps", name="ps")
            for m in range(24):
                nc.tensor.matmul(ps[:], w2_sb[m // 4][:, (m % 4) * 768 + f * 128:
                                                      (m % 4) * 768 + (f + 1) * 128],
                                 r_tiles[m][:],
                                 start=(m == 0), stop=(m == 23))
            ot = g_roll2.tile([128, 512], FP, tag="ot", name="ot")
            nc.vector.scalar_tensor_tensor(ot[:], ps[:], bias_sb[f][:, 4:5],
                                           xmid[f][:, g * 512:(g + 1) * 512],
                                           AluOpType.add, AluOpType.add)
            nc.sync.dma_start(d_out[f * 128:(f + 1) * 128, g * 512:(g + 1) * 512],
                              ot[:])
    mlp_es.close()
    es.close()


# ---------------------------------------------------------------------------
# host side
# ---------------------------------------------------------------------------

def _mycols(half):
    blocks = np.arange(8) * 2 + half
    return (blocks[:, None] * 128 + np.arange(128)[None, :]).reshape(-1)


def _prep_inputs(x, wq, bq, wk, bk, wv, bv, w_proj, b_proj, w1, b1, w2, b2,
                 g1, beta1, g2, beta2):
    x = np.asarray(x, np.float32)
    wq_f = np.ascontiguousarray(np.transpose(np.asarray(wq, np.float32), (1, 0, 2)).reshape(C, C))
    wk_f = np.ascontiguousarray(np.transpose(np.asarray(wk, np.float32), (1, 0, 2)).reshape(C, C))
    wv_f = np.ascontiguousarray(np.transpose(np.asarray(wv, np.float32), (1, 0, 2)).reshape(C, C))
    g1 = np.asarray(g1, np.float32); beta1 = np.asarray(beta1, np.float32)
    g2 = np.asarray(g2, np.float32); beta2 = np.asarray(beta2, np.float32)
    w1 = np.asarray(w1, np.float32); w2 = np.asarray(w2, np.float32)
    w_proj = np.asarray(w_proj, np.float32)

    wq_g = g1[:, None] * wq_f
    wk_g = g1[:, None] * wk_f
    wv_g = g1[:, None] * wv_f
    bq_f = beta1 @ wq_f + np.asarray(bq, np.float32).reshape(-1)
    bk_f = beta1 @ wk_f + np.asarray(bk, np.float32).reshape(-1)
    bv_f = beta1 @ wv_f + np.asarray(bv, np.float32).reshape(-1)
    w1_g = g2[:, None] * w1
    b1_f = beta2 @ w1 + np.asarray(b1, np.float32)

    wqkvp = np.concatenate([wq_g, wk_g, wv_g, w_proj], axis=1).astype(bf16)
    w1p = w1_g.astype(bf16)
    w2p = np.ascontiguousarray(
        w2.reshape(6, 4, 128, C).transpose(0, 2, 1, 3).reshape(C, 4 * C)).astype(bf16)

    biasp = np.zeros((C, 9), np.float32)
    biasp[:, 0] = bq_f
    biasp[:, 1] = bk_f
    biasp[:, 2] = bv_f
    biasp[:, 3] = np.asarray(b_proj, np.float32)
    biasp[:, 4] = np.asarray(b2, np.float32)
    biasp[:, 5:9] = b1_f.reshape(4, C).T
    bvrow = bv_f.reshape(1, C).astype(np.float32)

    tri = np.tril(np.ones((128, 128), np.float32)).T  # [s, q]: 1 iff s <= q
    in_maps = []
    for core in range(8):
        b, half = core // 2, core % 2
        xT = np.ascontiguousarray(x[b].T)
        xTm = np.ascontiguousarray(xT[:, _mycols(half)])
        masks = np.zeros((256, 128), np.float32)
        if half == 0:
            masks[0:128] = tri
            masks[128:256] = 0.0
        else:
            masks[0:128] = 1.0
            masks[128:256] = tri
        in_maps.append({
            "xT": xT, "xTm": xTm,
            "wqkvp": wqkvp, "w1p": w1p, "w2p": w2p,
            "biasp": biasp, "bvrow": bvrow,
            "masks": masks.astype(bf16),
        })
    return in_maps


def _assemble(results, dtype):
    out = np.empty((B, T, C), dtype)
    for core in range(8):
        b, half = core // 2, core % 2
        out[b, _mycols(half), :] = results[core]["outT"].T
    return out


def kernel(**inputs):
    in_maps = _prep_inputs(**inputs)
    if "nc" not in _cache:
        _cache["nc"] = _build()
    res = bass_utils.run_bass_kernel_spmd(_cache["nc"], in_maps,
                                          core_ids=list(range(8)))
    return _assemble(res.results, np.asarray(inputs["x"]).dtype)



# revision 14
# speedup vs baseline: 1.0670x; 1.0670x over previous
"""Transformer block (LN -> 12-head causal attention -> residual -> LN -> MLP
-> residual) for B=4, T=2048, C=768 on 8 trn2 NeuronCores.

Sharding: core = (batch, token-half). Each core handles one batch's K/V in
full and produces the final output for half the tokens (even or odd 128-token
blocks, which balances the causal-attention triangle). No collectives; all
per-core structural differences are carried in input *data* (host-gathered
xT_mine, causal-boundary mask tiles) so a single SPMD program runs on all 8
cores.

On-chip layout is feature-major ("transposed", [C, T]): LN statistics are
computed with ones-vector matmuls on the tensor engine, Q^T/K^T land directly
in the layout attention wants, softmax runs shift-exp (constant shift, exact
softmax) with the row-sum fused into the P@V matmul via a ones column
appended to V, and residuals/bias adds ride the PSUM->SBUF copies.
"""

import math
import os
import sys

for _p in ("/opt/trn_rl_repo", "/root/.axon_site/_ro/trn_rl_repo"):
    if os.path.isdir(_p) and _p not in sys.path:
        sys.path.append(_p)

import numpy as np
import ml_dtypes

import concourse.bacc as bacc
import concourse.tile as tile
import concourse.mybir as mybir
from concourse import bass_utils
from concourse.alu_op_type import AluOpType
from concourse.tile_rust import add_dep_helper

BF = mybir.dt.bfloat16
FP = mybir.dt.float32
F32R = mybir.dt.float32r
AF = mybir.ActivationFunctionType

B, T, C, H, HD = 4, 2048, 768, 12, 64
EPS = 1e-5
SHIFT = 40.0  # constant softmax shift: exp(s - SHIFT); exact softmax
NP = C // 128  # 6 feature partition-tiles
NT = T // 128  # 16 token blocks
TM = T // 2    # 1024 tokens owned per core
NB = 16        # token blocks
bf16 = ml_dtypes.bfloat16

_cache = {}


def _build(debug=False):
    nc = bacc.Bacc("TRN2", target_bir_lowering=False, debug=False)
    d_xT = nc.dram_tensor("xT", [C, T], FP, kind="ExternalInput").ap()
    d_xTm = nc.dram_tensor("xTm", [C, TM], FP, kind="ExternalInput").ap()
    d_wqkvp = nc.dram_tensor("wqkvp", [C, 4 * C], BF, kind="ExternalInput").ap()
    d_w1 = nc.dram_tensor("w1p", [C, 4 * C], BF, kind="ExternalInput").ap()
    d_w2 = nc.dram_tensor("w2p", [C, 4 * C], BF, kind="ExternalInput").ap()
    d_bias = nc.dram_tensor("biasp", [C, 9], FP, kind="ExternalInput").ap()
    d_bvrow = nc.dram_tensor("bvrow", [1, C], FP, kind="ExternalInput").ap()
    d_masks = nc.dram_tensor("masks", [256, 128], BF, kind="ExternalInput").ap()
    d_out = nc.dram_tensor("outT", [C, TM], FP, kind="ExternalOutput").ap()
    if debug is True:
        debug = ["h", "hm", "KT", "QT", "V", "attnT", "xmid"]
    debug = debug or []
    dbg = {}
    if "h" in debug:
        dbg["h"] = nc.dram_tensor("dbg_h", [C, T], BF, kind="ExternalOutput").ap()
    if "hm" in debug:
        dbg["hm"] = nc.dram_tensor("dbg_hm", [C, TM], BF, kind="ExternalOutput").ap()
    if "KT" in debug:
        dbg["KT"] = nc.dram_tensor("dbg_KT", [C, T], BF, kind="ExternalOutput").ap()
    if "QT" in debug:
        dbg["QT"] = nc.dram_tensor("dbg_QT", [C, TM], BF, kind="ExternalOutput").ap()
    if "V" in debug:
        dbg["V"] = nc.dram_tensor("dbg_V", [T, H * 65], BF, kind="ExternalOutput").ap()
    if "attnT" in debug:
        dbg["attnT"] = nc.dram_tensor("dbg_attnT", [C, TM], BF, kind="ExternalOutput").ap()
    if "xmid" in debug:
        dbg["xmid"] = nc.dram_tensor("dbg_xmid", [C, TM], FP, kind="ExternalOutput").ap()

    with tile.TileContext(nc) as tc:
        _body(nc, tc, d_xT, d_xTm, d_wqkvp, d_w1, d_w2, d_bias, d_bvrow,
              d_masks, d_out, dbg)
    nc.compile()
    return nc


def _ln_stats_rows(nc, tc, small, sps, x_dram, ncols, ones_bf, eps_c, sq_of, tag):
    """Stream x (feature-major, fp32, [C, ncols]) from DRAM; return bf16
    broadcast tiles a_bc, c_bc ([128, ncols]) with h = x*a + c == LN(x)."""
    ntg = ncols // 512
    stats = [sps.tile([33, 512], FP, tag=f"{tag}st{g}", name=f"{tag}st{g}") for g in range(ntg)]
    for ci in range(NP):
        for g in range(ntg):
            csl = slice(g * 512, (g + 1) * 512)
            xt = sq_of["roll"].tile([128, 512], FP, tag="xr", name="xr")
            nc.sync.dma_start(xt[:], x_dram[ci * 128:(ci + 1) * 128, csl])
            xb = sq_of["roll"].tile([128, 512], BF, tag="xb", name="xb")
            nc.vector.tensor_copy(xb[:], xt[:])
            sq = sq_of["roll"].tile([128, 512], BF, tag="sq", name="sq")
            nc.vector.tensor_mul(sq[:], xb[:], xb[:])
            nc.tensor.matmul(stats[g][0:1, :], ones_bf[:], xb[:],
                             start=(ci == 0), stop=(ci == NP - 1),
                             skip_group_check=True)
            nc.tensor.matmul(stats[g][32:33, :], ones_bf[:], sq[:],
                             start=(ci == 0), stop=(ci == NP - 1),
                             skip_group_check=True)
    a_bc = sq_of["bc"].tile([128, ncols], BF, tag=f"{tag}abc", name=f"{tag}abc")
    c_bc = sq_of["bc"].tile([128, ncols], BF, tag=f"{tag}cbc", name=f"{tag}cbc")
    for g in range(ntg):
        sl = slice(g * 512, (g + 1) * 512)
        mu = small.tile([1, 512], FP, tag="mu", name="mu")
        nc.scalar.mul(mu[:], stats[g][0:1, :], 1.0 / C)
        m2 = small.tile([1, 512], FP, tag="m2", name="m2")
        nc.scalar.mul(m2[:], stats[g][32:33, :], 1.0 / C)
        var = small.tile([1, 512], FP, tag="va", name="va")
        # var = m2 - mu*mu
        nc.vector.tensor_mul(var[:], mu[:], mu[:])
        nc.vector.tensor_sub(var[:], m2[:], var[:])
        std = small.tile([1, 512], FP, tag="sd", name="sd")
        nc.scalar.activation(std[:], var[:], AF.Sqrt, bias=eps_c[0:1, 0:1])
        a5 = small.tile([1, 512], FP, tag="a5", name="a5")
        nc.vector.reciprocal(a5[:], std[:])
        # c = -mu * a
        c5 = small.tile([1, 512], FP, tag="c5", name="c5")
        nc.vector.tensor_mul(c5[:], mu[:], a5[:])
        nc.vector.tensor_scalar_mul(c5[:], c5[:], -1.0)
        a5b = small.tile([1, 512], BF, tag="a5b", name="a5b")
        nc.vector.tensor_copy(a5b[:], a5[:])
        c5b = small.tile([1, 512], BF, tag="c5b", name="c5b")
        nc.vector.tensor_copy(c5b[:], c5[:])
        nc.gpsimd.partition_broadcast(a_bc[:, sl], a5b[:])
        nc.gpsimd.partition_broadcast(c_bc[:, sl], c5b[:])
    return a_bc, c_bc


def _body(nc, tc, d_xT, d_xTm, d_wqkvp, d_w1, d_w2, d_bias, d_bvrow,
          d_masks, d_out, dbg={}):
    from contextlib import ExitStack

    es = ExitStack()
    g_const = es.enter_context(tc.tile_pool(name="const", bufs=1))
    g_xmid = es.enter_context(tc.tile_pool(name="xmid", bufs=1))
    xmid = [g_xmid.tile([128, TM], FP, tag=f"xm{i}", name=f"xm{i}") for i in range(NP)]
    w_es = ExitStack()
    kqv_stack = ExitStack()
    g_w = w_es.enter_context(tc.tile_pool(name="wqkvp", bufs=1))
    g_kqv = kqv_stack.enter_context(tc.tile_pool(name="kqv", bufs=1))

    # ---- constants ----
    ones_bf = g_const.tile([128, 1], BF, tag="ones_bf", name="ones_bf")
    nc.vector.memset(ones_bf[:], 1.0)
    eps_c = g_const.tile([128, 1], FP, tag="eps_c", name="eps_c")
    nc.vector.memset(eps_c[:], EPS)
    shift_c = g_const.tile([128, 1], FP, tag="shift_c", name="shift_c")
    nc.vector.memset(shift_c[:], -SHIFT)
    bias_sb = [g_const.tile([128, 9], FP, tag=f"bias{f}", name=f"bias{f}") for f in range(NP)]
    for f in range(NP):
        nc.sync.dma_start(bias_sb[f][:], d_bias[f * 128:(f + 1) * 128, :])
    mask_a = g_const.tile([128, 128], BF, tag="mask_a", name="mask_a")
    nc.sync.dma_start(mask_a[:], d_masks[0:128, :])
    mask_b = g_const.tile([128, 128], BF, tag="mask_b", name="mask_b")
    nc.sync.dma_start(mask_b[:], d_masks[128:256, :])
    bv_row = g_const.tile([1, C], FP, tag="bv_row", name="bv_row")
    nc.sync.dma_start(bv_row[:], d_bvrow[:])
    bv_rb = g_const.tile([1, C], BF, tag="bv_rb", name="bv_rb")
    nc.vector.tensor_copy(bv_rb[:], bv_row[:])
    bv_bc = g_const.tile([128, C], BF, tag="bv_bc", name="bv_bc")
    nc.gpsimd.partition_broadcast(bv_bc[:], bv_rb[:])

    # ---- weights for attention part ----
    w_sb = [g_w.tile([128, 4 * C], BF, tag=f"w{c}", name=f"w{c}") for c in range(NP)]
    for c in range(NP):
        nc.sync.dma_start(w_sb[c][:], d_wqkvp[c * 128:(c + 1) * 128, :])

    # ---- persistent activation storage ----
    KT = [g_kqv.tile([128, T], BF, tag=f"KT{i}", name=f"KT{i}") for i in range(NP)]
    QT = [g_kqv.tile([128, TM], BF, tag=f"QT{i}", name=f"QT{i}") for i in range(NP)]
    Vsb = [g_kqv.tile([128, H * 65], BF, tag=f"V{t}", name=f"V{t}") for t in range(NT)]

    # ================= LN1 + QKV =================
    ln_es = ExitStack()
    g_roll = ln_es.enter_context(tc.tile_pool(name="lnroll", bufs=2))
    g_bc = ln_es.enter_context(tc.tile_pool(name="lnbc", bufs=1))
    g_small = ln_es.enter_context(tc.tile_pool(name="lnsmall", bufs=1))
    g_h = ln_es.enter_context(tc.tile_pool(name="hpool", bufs=1))
    pools = {"roll": g_roll, "bc": g_bc}
    sps_es = ExitStack()
    sps = sps_es.enter_context(tc.tile_pool(name="statps", bufs=1, space="PSUM"))
    a_bc, c_bc = _ln_stats_rows(nc, tc, g_small, sps, d_xT, T, ones_bf, eps_c,
                                pools, "f")
    am_bc, cm_bc = _ln_stats_rows(nc, tc, g_small, sps, d_xTm, TM, ones_bf, eps_c,
                                  pools, "m")
    sps_es.close()

    h = [g_h.tile([128, T], BF, tag=f"h{c}", name=f"h{c}") for c in range(NP)]
    hm = [g_h.tile([128, TM], BF, tag=f"hm{c}", name=f"hm{c}") for c in range(NP)]
    for c in range(NP):
        for ch in range(4):
            csl = slice(ch * 512, (ch + 1) * 512)
            xt = g_roll.tile([128, 512], FP, tag="xr", name="xr")
            nc.sync.dma_start(xt[:], d_xT[c * 128:(c + 1) * 128, csl])
            tmp = g_roll.tile([128, 512], FP, tag="h_tmp", name="h_tmp")
            nc.vector.tensor_mul(tmp[:], xt[:], a_bc[:, csl])
            nc.vector.tensor_add(h[c][:, csl], tmp[:], c_bc[:, csl])
        for ch in range(2):
            csl = slice(ch * 512, (ch + 1) * 512)
            xtm = g_roll.tile([128, 512], FP, tag="xr", name="xr")
            nc.sync.dma_start(xtm[:], d_xTm[c * 128:(c + 1) * 128, csl])
            tmpm = g_roll.tile([128, 512], FP, tag="h_tmp", name="h_tmp")
            nc.vector.tensor_mul(tmpm[:], xtm[:], am_bc[:, csl])
            nc.vector.tensor_add(hm[c][:, csl], tmpm[:], cm_bc[:, csl])

    qkv_es = ExitStack()
    gps = qkv_es.enter_context(tc.tile_pool(name="gemmps", bufs=3, space="PSUM"))

    # K^T [C, T]: lhsT = wk tile, rhs = h
    for f in range(NP):
        for g in range(T // 512):
            ps = gps.tile([128, 512], FP, tag="ps", name="ps")
            for c in range(NP):
                nc.tensor.matmul(ps[:], w_sb[c][:, C + f * 128:C + (f + 1) * 128],
                                 h[c][:, g * 512:(g + 1) * 512],
                                 start=(c == 0), stop=(c == NP - 1))
            nc.vector.tensor_scalar_add(KT[f][:, g * 512:(g + 1) * 512], ps[:],
                                        bias_sb[f][:, 1:2])
    # Q^T [C, TM] from h_mine
    for f in range(NP):
        for g in range(TM // 512):
            ps = gps.tile([128, 512], FP, tag="ps", name="ps")
            for c in range(NP):
                nc.tensor.matmul(ps[:], w_sb[c][:, f * 128:(f + 1) * 128],
                                 hm[c][:, g * 512:(g + 1) * 512],
                                 start=(c == 0), stop=(c == NP - 1))
            nc.vector.tensor_scalar_add(QT[f][:, g * 512:(g + 1) * 512], ps[:],
                                        bias_sb[f][:, 0:1])
    # V natural [T, C] (+ ones col per head): lhsT = h tile, rhs = wv
    for t in range(NT):
        v3 = Vsb[t][:].rearrange("p (h d) -> p h d", d=65)
        nc.vector.memset(v3[:, :, 64:65], 1.0)
        for fs in range(2):
            n = 512 if fs == 0 else 256
            nh = n // 64
            ps = gps.tile([128, 512], FP, tag="ps", name="ps")
            for c in range(NP):
                nc.tensor.matmul(ps[:, 0:n], h[c][:, t * 128:(t + 1) * 128],
                                 w_sb[c][:, 2 * C + fs * 512:2 * C + fs * 512 + n],
                                 start=(c == 0), stop=(c == NP - 1))
            # scatter into per-head 65-wide slots with bias add
            nc.vector.scalar_tensor_tensor(
                v3[:, fs * 8:fs * 8 + nh, 0:64],
                ps[:, 0:n].rearrange("p (h d) -> p h d", d=64),
                0.0,
                bv_bc[:, fs * 512:fs * 512 + n].rearrange("p (h d) -> p h d", d=64),
                AluOpType.add, AluOpType.add)
    for c in range(NP):
        if "h" in dbg:
            nc.sync.dma_start(dbg["h"][c * 128:(c + 1) * 128, :], h[c][:])
        if "hm" in dbg:
            nc.sync.dma_start(dbg["hm"][c * 128:(c + 1) * 128, :], hm[c][:])
        if "KT" in dbg:
            nc.sync.dma_start(dbg["KT"][c * 128:(c + 1) * 128, :], KT[c][:])
        if "QT" in dbg:
            nc.sync.dma_start(dbg["QT"][c * 128:(c + 1) * 128, :], QT[c][:])
    if "V" in dbg:
        for t in range(NT):
            nc.sync.dma_start(dbg["V"][t * 128:(t + 1) * 128, :], Vsb[t][:])
    ln_es.close()
    qkv_es.close()

    attnT_es = ExitStack()
    g_attnT = attnT_es.enter_context(tc.tile_pool(name="attnT", bufs=1))
    attnT = [g_attnT.tile([128, TM], BF, tag=f"aT{i}", name=f"aT{i}") for i in range(NP)]

    # ================= attention =================
    att_es = ExitStack()
    g_wei = att_es.enter_context(tc.tile_pool(name="wei", bufs=3))
    g_asc = att_es.enter_context(tc.tile_pool(name="ascratch", bufs=4))
    g_aU = att_es.enter_context(tc.tile_pool(name="attnU", bufs=2))
    g_rb = att_es.enter_context(tc.tile_pool(name="rbpool", bufs=2))
    ps_s_pool = att_es.enter_context(tc.tile_pool(name="sps", bufs=2, space="PSUM"))
    ps_a_pool = att_es.enter_context(tc.tile_pool(name="aps", bufs=2, space="PSUM"))

    for hh in range(H):
        ht, hp = hh // 2, (hh % 2) * 64
        for g in range(2):
            smax = 8 + 8 * g
            pa = ps_a_pool.tile([65, 512], FP, tag="pa", name="pa")
            for sb in range(smax):
                jmin = max(0, math.ceil((sb - 1 - 8 * g) / 2))
                c0 = jmin * 128
                ps = ps_s_pool.tile([128, 512], FP, tag="ps", name="ps")
                nc.tensor.matmul(ps[:, c0:512],
                                 KT[ht][hp:hp + 64, sb * 128:(sb + 1) * 128],
                                 QT[ht][hp:hp + 64, g * 512 + c0:(g + 1) * 512],
                                 start=True, stop=True)
                wei = g_wei.tile([128, 512], BF, tag="wei", name="wei")
                nc.scalar.activation(wei[:, c0:512], ps[:, c0:512], AF.Exp,
                                     bias=shift_c[:])
                if (sb - 8 * g) % 2 == 0:
                    ja = (sb - 8 * g) // 2
                    if 0 <= ja < 4:
                        nc.vector.tensor_mul(wei[:, ja * 128:(ja + 1) * 128],
                                             wei[:, ja * 128:(ja + 1) * 128],
                                             mask_a[:])
                else:
                    jb = (sb - 1 - 8 * g) // 2
                    if 0 <= jb < 4:
                        nc.vector.tensor_mul(wei[:, jb * 128:(jb + 1) * 128],
                                             wei[:, jb * 128:(jb + 1) * 128],
                                             mask_b[:])
                nc.tensor.matmul(pa[:, c0:512], Vsb[sb][:, hh * 65:(hh + 1) * 65],
                                 wei[:, c0:512], start=(sb == 0),
                                 stop=(sb == smax - 1), skip_group_check=True)
            sumrow = g_asc.tile([1, 512], FP, tag="sumrow", name="sumrow")
            nc.vector.tensor_copy(sumrow[:], pa[64:65, :])
            recip = g_asc.tile([1, 512], FP, tag="recip", name="recip")
            scr = g_asc.tile([1, 512], FP, tag="scr", name="scr")
            nc.vector.reciprocal_approx_accurate(recip[:], sumrow[:], scr[:])
            rb = g_rb.tile([64, 512], FP, tag="rb", name="rb")
            nc.gpsimd.partition_broadcast(rb[:], recip[:])
            nc.vector.tensor_mul(attnT[ht][hp:hp + 64, g * 512:(g + 1) * 512],
                                 pa[0:64, :], rb[:])
    if "attnT" in dbg:
        for c in range(NP):
            nc.sync.dma_start(dbg["attnT"][c * 128:(c + 1) * 128, :], attnT[c][:])
    att_es.close()

    # ================= proj + residual =================
    proj_insts = []
    proj_es = ExitStack()
    gps = proj_es.enter_context(tc.tile_pool(name="pps", bufs=3, space="PSUM"))
    g_pr = proj_es.enter_context(tc.tile_pool(name="projroll", bufs=2))
    for f in range(NP):
        for g in range(TM // 512):
            ps = gps.tile([128, 512], FP, tag="ps", name="ps")
            for c in range(NP):
                nc.tensor.matmul(ps[:], w_sb[c][:, 3 * C + f * 128:3 * C + (f + 1) * 128],
                                 attnT[c][:, g * 512:(g + 1) * 512],
                                 start=(c == 0), stop=(c == NP - 1))
            xr = g_pr.tile([128, 512], FP, tag="xr", name="xr")
            nc.sync.dma_start(xr[:], d_xTm[f * 128:(f + 1) * 128,
                                           g * 512:(g + 1) * 512])
            pi = nc.vector.scalar_tensor_tensor(
                xmid[f][:, g * 512:(g + 1) * 512], ps[:], bias_sb[f][:, 3:4],
                xr[:], AluOpType.add, AluOpType.add)
            proj_insts.append(pi)
    if "xmid" in dbg:
        for c in range(NP):
            nc.sync.dma_start(dbg["xmid"][c * 128:(c + 1) * 128, :], xmid[c][:])
    proj_es.close()
    attnT_es.close()
    kqv_stack.close()
    w_es.close()

    # ================= LN2 + MLP =================
    mlp_es = ExitStack()
    g_w1 = mlp_es.enter_context(tc.tile_pool(name="w1p", bufs=1))
    g_w2 = mlp_es.enter_context(tc.tile_pool(name="w2p", bufs=1))
    g_h2 = mlp_es.enter_context(tc.tile_pool(name="h2p", bufs=1))
    g_r = mlp_es.enter_context(tc.tile_pool(name="rp", bufs=2))
    g_roll2 = mlp_es.enter_context(tc.tile_pool(name="mlproll", bufs=2))
    g_bc2 = mlp_es.enter_context(tc.tile_pool(name="mlpbc", bufs=1))
    g_small2 = mlp_es.enter_context(tc.tile_pool(name="mlpsmall", bufs=1))
    sps2 = mlp_es.enter_context(tc.tile_pool(name="statps2", bufs=1, space="PSUM"))
    gps2 = mlp_es.enter_context(tc.tile_pool(name="mps", bufs=3, space="PSUM"))

    w1_sb = [g_w1.tile([128, 4 * C], BF, tag=f"w1_{c}", name=f"w1_{c}") for c in range(NP)]
    w2_sb = [g_w2.tile([128, 4 * C], BF, tag=f"w2_{c}", name=f"w2_{c}") for c in range(NP)]
    for c in range(NP):
        i1 = nc.sync.dma_start(w1_sb[c][:], d_w1[c * 128:(c + 1) * 128, :])
        i2 = nc.sync.dma_start(w2_sb[c][:], d_w2[c * 128:(c + 1) * 128, :])
        for pi in proj_insts:
            add_dep_helper(i1.ins, pi.ins, reason="mlp w-load after proj")
            add_dep_helper(i2.ins, pi.ins, reason="mlp w-load after proj")

    # LN2 stats from xmid (already in SBUF)
    ntg = TM // 512
    stats = [sps2.tile([33, 512], FP, tag=f"st2{g}", name=f"st2{g}") for g in range(ntg)]
    for ci in range(NP):
        xb = g_roll2.tile([128, TM], BF, tag="xb2", name="xb2")
        nc.vector.tensor_copy(xb[:], xmid[ci][:])
        sq = g_roll2.tile([128, TM], BF, tag="sq2", name="sq2")
        nc.vector.tensor_mul(sq[:], xb[:], xb[:])
        for g in range(ntg):
            nc.tensor.matmul(stats[g][0:1, :], ones_bf[:],
                             xb[:, g * 512:(g + 1) * 512],
                             start=(ci == 0), stop=(ci == NP - 1),
                             skip_group_check=True)
            nc.tensor.matmul(stats[g][32:33, :], ones_bf[:],
                             sq[:, g * 512:(g + 1) * 512],
                             start=(ci == 0), stop=(ci == NP - 1),
                             skip_group_check=True)
    a_bc2 = g_bc2.tile([128, TM], BF, tag="a2bc", name="a2bc")
    c_bc2 = g_bc2.tile([128, TM], BF, tag="c2bc", name="c2bc")
    for g in range(ntg):
        sl = slice(g * 512, (g + 1) * 512)
        mu = g_small2.tile([1, 512], FP, tag="mu2", name="mu2")
        nc.scalar.mul(mu[:], stats[g][0:1, :], 1.0 / C)
        m2 = g_small2.tile([1, 512], FP, tag="m22", name="m22")
        nc.scalar.mul(m2[:], stats[g][32:33, :], 1.0 / C)
        var = g_small2.tile([1, 512], FP, tag="va2", name="va2")
        nc.vector.tensor_mul(var[:], mu[:], mu[:])
        nc.vector.tensor_sub(var[:], m2[:], var[:])
        std = g_small2.tile([1, 512], FP, tag="sd2", name="sd2")
        nc.scalar.activation(std[:], var[:], AF.Sqrt, bias=eps_c[0:1, 0:1])
        a5 = g_small2.tile([1, 512], FP, tag="a52", name="a52")
        nc.vector.reciprocal(a5[:], std[:])
        c5 = g_small2.tile([1, 512], FP, tag="c52", name="c52")
        nc.vector.tensor_mul(c5[:], mu[:], a5[:])
        nc.vector.tensor_scalar_mul(c5[:], c5[:], -1.0)
        a5b = g_small2.tile([1, 512], BF, tag="a5b2", name="a5b2")
        nc.vector.tensor_copy(a5b[:], a5[:])
        c5b = g_small2.tile([1, 512], BF, tag="c5b2", name="c5b2")
        nc.vector.tensor_copy(c5b[:], c5[:])
        nc.gpsimd.partition_broadcast(a_bc2[:, sl], a5b[:])
        nc.gpsimd.partition_broadcast(c_bc2[:, sl], c5b[:])

    h2 = [g_h2.tile([128, TM], BF, tag=f"h2_{c}", name=f"h2_{c}") for c in range(NP)]
    for c in range(NP):
        tmp = g_roll2.tile([128, TM], FP, tag="h2tmp", name="h2tmp")
        nc.vector.tensor_mul(tmp[:], xmid[c][:], a_bc2[:])
        nc.vector.tensor_add(h2[c][:], tmp[:], c_bc2[:])

    for g in range(TM // 512):
        r_tiles = []
        for m in range(24):
            ps = gps2.tile([128, 512], FP, tag="ps", name="ps")
            for c in range(NP):
                nc.tensor.matmul(ps[:], w1_sb[c][:, m * 128:(m + 1) * 128],
                                 h2[c][:, g * 512:(g + 1) * 512],
                                 start=(c == 0), stop=(c == NP - 1))
            r = g_r.tile([128, 512], BF, tag=f"r{m}", name=f"r{m}")
            nc.vector.tensor_scalar(r[:], ps[:], bias_sb[m % 6][:, 5 + m // 6:6 + m // 6],
                                    0.0, AluOpType.add, AluOpType.max)
            r_tiles.append(r)
        for f in range(NP):
            ps = gps2.tile([128, 512], FP, tag="ps", name="ps")
            for m in range(24):
                nc.tensor.matmul(ps[:], w2_sb[m // 4][:, (m % 4) * 768 + f * 128:
                                                      (m % 4) * 768 + (f + 1) * 128],
                                 r_tiles[m][:],
                                 start=(m == 0), stop=(m == 23))
            ot = g_roll2.tile([128, 512], FP, tag="ot", name="ot")
            nc.vector.scalar_tensor_tensor(ot[:], ps[:], bias_sb[f][:, 4:5],
                                           xmid[f][:, g * 512:(g + 1) * 512],
                                           AluOpType.add, AluOpType.add)
            nc.sync.dma_start(d_out[f * 128:(f + 1) * 128, g * 512:(g + 1) * 512],
                              ot[:])
    mlp_es.close()
    es.close()


# ---------------------------------------------------------------------------
# host side
# ---------------------------------------------------------------------------

def _mycols(half):
    blocks = np.arange(8) * 2 + half
    return (blocks[:, None] * 128 + np.arange(128)[None, :]).reshape(-1)


def _prep_inputs(x, wq, bq, wk, bk, wv, bv, w_proj, b_proj, w1, b1, w2, b2,
                 g1, beta1, g2, beta2):
    x = np.asarray(x, np.float32)
    wq_f = np.ascontiguousarray(np.transpose(np.asarray(wq, np.float32), (1, 0, 2)).reshape(C, C))
    wk_f = np.ascontiguousarray(np.transpose(np.asarray(wk, np.float32), (1, 0, 2)).reshape(C, C))
    wv_f = np.ascontiguousarray(np.transpose(np.asarray(wv, np.float32), (1, 0, 2)).reshape(C, C))
    g1 = np.asarray(g1, np.float32); beta1 = np.asarray(beta1, np.float32)
    g2 = np.asarray(g2, np.float32); beta2 = np.asarray(beta2, np.float32)
    w1 = np.asarray(w1, np.float32); w2 = np.asarray(w2, np.float32)
    w_proj = np.asarray(w_proj, np.float32)

    wq_g = g1[:, None] * wq_f
    wk_g = g1[:, None] * wk_f
    wv_g = g1[:, None] * wv_f
    bq_f = beta1 @ wq_f + np.asarray(bq, np.float32).reshape(-1)
    bk_f = beta1 @ wk_f + np.asarray(bk, np.float32).reshape(-1)
    bv_f = beta1 @ wv_f + np.asarray(bv, np.float32).reshape(-1)
    w1_g = g2[:, None] * w1
    b1_f = beta2 @ w1 + np.asarray(b1, np.float32)

    wqkvp = np.concatenate([wq_g, wk_g, wv_g, w_proj], axis=1).astype(bf16)
    w1p = w1_g.astype(bf16)
    w2p = np.ascontiguousarray(
        w2.reshape(6, 4, 128, C).transpose(0, 2, 1, 3).reshape(C, 4 * C)).astype(bf16)

    biasp = np.zeros((C, 9), np.float32)
    biasp[:, 0] = bq_f
    biasp[:, 1] = bk_f
    biasp[:, 2] = bv_f
    biasp[:, 3] = np.asarray(b_proj, np.float32)
    biasp[:, 4] = np.asarray(b2, np.float32)
    biasp[:, 5:9] = b1_f.reshape(4, C).T
    bvrow = bv_f.reshape(1, C).astype(np.float32)

    tri = np.tril(np.ones((128, 128), np.float32)).T  # [s, q]: 1 iff s <= q
    in_maps = []
    for core in range(8):
        b, half = core // 2, core % 2
        xT = np.ascontiguousarray(x[b].T)
        xTm = np.ascontiguousarray(xT[:, _mycols(half)])
        masks = np.zeros((256, 128), np.float32)
        if half == 0:
            masks[0:128] = tri
            masks[128:256] = 0.0
        else:
            masks[0:128] = 1.0
            masks[128:256] = tri
        in_maps.append({
            "xT": xT, "xTm": xTm,
            "wqkvp": wqkvp, "w1p": w1p, "w2p": w2p,
            "biasp": biasp, "bvrow": bvrow,
            "masks": masks.astype(bf16),
        })
    return in_maps


def _assemble(results, dtype):
    out = np.empty((B, T, C), dtype)
    for core in range(8):
        b, half = core // 2, core % 2
        out[b, _mycols(half), :] = results[core]["outT"].T
    return out


def kernel(**inputs):
    in_maps = _prep_inputs(**inputs)
    if "nc" not in _cache:
        _cache["nc"] = _build()
    res = bass_utils.run_bass_kernel_spmd(_cache["nc"], in_maps,
                                          core_ids=list(range(8)))
    return _assemble(res.results, np.asarray(inputs["x"]).dtype)



# revision 29
# speedup vs baseline: 1.3285x; 1.2450x over previous
"""Transformer block (LN -> 12-head causal attention -> residual -> LN -> MLP
-> residual) for B=4, T=2048, C=768 on 8 trn2 NeuronCores.

Sharding: core = (batch, token-half). Each core handles one batch's K/V in
full and produces the final output for half the tokens (even or odd 128-token
blocks, which balances the causal-attention triangle). No collectives; all
per-core structural differences are carried in input *data* (host-gathered
xTm, causal-boundary mask tiles) so a single SPMD program runs on all 8
cores.

On-chip layout is feature-major ("transposed", [C, T]). LN statistics are
computed with ones-vector matmuls on the tensor engine and the LN1+QKV
pipeline is interleaved per 512-token group so the PE never starves.
Attention processes all 1024 owned query columns per head in one pass
(1024-wide exp tiles halve the scalar-engine instruction count); softmax
row-sums ride a ones column appended to V and are inverted with the fast
Newton-Raphson reciprocal.
"""

import math
import os
import sys

for _p in ("/opt/trn_rl_repo", "/root/.axon_site/_ro/trn_rl_repo"):
    if os.path.isdir(_p) and _p not in sys.path:
        sys.path.append(_p)

import numpy as np
import ml_dtypes

import concourse.bacc as bacc
import concourse.tile as tile
import concourse.mybir as mybir
from concourse import bass_utils
from concourse.alu_op_type import AluOpType
from concourse.tile_rust import add_dep_helper

BF = mybir.dt.bfloat16
FP = mybir.dt.float32
AF = mybir.ActivationFunctionType

B, T, C, H, HD = 4, 2048, 768, 12, 64
EPS = 1e-5
SHIFT = 40.0  # constant softmax shift: exp(s - SHIFT); exact softmax
NP = C // 128  # 6 feature partition-tiles
NT = T // 128  # 16 token blocks
TM = T // 2    # 1024 tokens owned per core
NG = 4         # 512-token groups
bf16 = ml_dtypes.bfloat16

_cache = {}


def _build(debug=False):
    nc = bacc.Bacc("TRN2", target_bir_lowering=False, debug=False)
    d_xT = nc.dram_tensor("xT", [C, T], FP, kind="ExternalInput").ap()
    d_xTm = nc.dram_tensor("xTm", [C, TM], FP, kind="ExternalInput").ap()
    d_wqkvp = nc.dram_tensor("wqkvp", [C, 4 * C], BF, kind="ExternalInput").ap()
    d_w1 = nc.dram_tensor("w1p", [C, 4 * C], BF, kind="ExternalInput").ap()
    d_w2 = nc.dram_tensor("w2p", [C, 4 * C], BF, kind="ExternalInput").ap()
    d_bias = nc.dram_tensor("biasp", [C, 9], FP, kind="ExternalInput").ap()
    d_bvrow = nc.dram_tensor("bvrow", [1, C], FP, kind="ExternalInput").ap()
    d_masks = nc.dram_tensor("masks", [256, 128], BF, kind="ExternalInput").ap()
    d_out = nc.dram_tensor("outT", [C, TM], FP, kind="ExternalOutput").ap()
    if debug is True:
        debug = ["h", "hm", "KT", "QT", "V", "attnT", "xmid"]
    debug = debug or []
    dbg = {}
    if "h" in debug:
        dbg["h"] = nc.dram_tensor("dbg_h", [C, T], BF, kind="ExternalOutput").ap()
    if "hm" in debug:
        dbg["hm"] = nc.dram_tensor("dbg_hm", [C, TM], BF, kind="ExternalOutput").ap()
    if "KT" in debug:
        dbg["KT"] = nc.dram_tensor("dbg_KT", [C, T], BF, kind="ExternalOutput").ap()
    if "QT" in debug:
        dbg["QT"] = nc.dram_tensor("dbg_QT", [C, TM], BF, kind="ExternalOutput").ap()
    if "V" in debug:
        dbg["V"] = nc.dram_tensor("dbg_V", [T, H * 65], BF, kind="ExternalOutput").ap()
    if "attnT" in debug:
        dbg["attnT"] = nc.dram_tensor("dbg_attnT", [C, TM], BF, kind="ExternalOutput").ap()
    if "xmid" in debug:
        dbg["xmid"] = nc.dram_tensor("dbg_xmid", [C, TM], FP, kind="ExternalOutput").ap()

    with tile.TileContext(nc) as tc:
        _body(nc, tc, d_xT, d_xTm, d_wqkvp, d_w1, d_w2, d_bias, d_bvrow,
              d_masks, d_out, dbg)
    nc.compile()
    return nc


def _ln_smalls(nc, small, stats, ncols, eps_c, a_dst, c_dst):
    """From accumulated [33, ncols] stats (row 0 = sum, row 32 = sumsq),
    produce bf16 [1, ncols] rows a5b (1/std) and c5b (-mu/std)."""
    mu = small.tile([1, ncols], FP, tag=f"mu{ncols}", name="mu")
    nc.scalar.mul(mu[:], stats[0:1, :], 1.0 / C)
    m2 = small.tile([1, ncols], FP, tag=f"m2{ncols}", name="m2")
    nc.scalar.mul(m2[:], stats[32:33, :], 1.0 / C)
    var = small.tile([1, ncols], FP, tag=f"va{ncols}", name="va")
    nc.vector.tensor_mul(var[:], mu[:], mu[:])
    nc.vector.tensor_sub(var[:], m2[:], var[:])
    std = small.tile([1, ncols], FP, tag=f"sd{ncols}", name="sd")
    nc.scalar.activation(std[:], var[:], AF.Sqrt, bias=eps_c[0:1, 0:1])
    a5 = small.tile([1, ncols], FP, tag=f"a5{ncols}", name="a5")
    nc.vector.reciprocal_approx_accurate(a5[:], std[:], var[:])
    c5 = small.tile([1, ncols], FP, tag=f"c5{ncols}", name="c5")
    nc.vector.scalar_tensor_tensor(c5[:], mu[:], -1.0, a5[:],
                                   AluOpType.mult, AluOpType.mult)
    nc.vector.tensor_copy(a_dst[:], a5[:])
    nc.vector.tensor_copy(c_dst[:], c5[:])


def _body(nc, tc, d_xT, d_xTm, d_wqkvp, d_w1, d_w2, d_bias, d_bvrow,
          d_masks, d_out, dbg={}):
    from contextlib import ExitStack

    es = ExitStack()
    g_const = es.enter_context(tc.tile_pool(name="const", bufs=1))
    g_xmid = es.enter_context(tc.tile_pool(name="xmid", bufs=1))
    xmid = [g_xmid.tile([128, TM], FP, tag=f"xm{i}", name=f"xm{i}") for i in range(NP)]
    w_es = ExitStack()
    kqv_stack = ExitStack()
    g_w = w_es.enter_context(tc.tile_pool(name="wqkvp", bufs=1))
    g_kqv = kqv_stack.enter_context(tc.tile_pool(name="kqv", bufs=1))

    # ---- constants ----
    ones_bf = g_const.tile([128, 1], BF, tag="ones_bf", name="ones_bf")
    nc.vector.memset(ones_bf[:], 1.0)
    eps_c = g_const.tile([128, 1], FP, tag="eps_c", name="eps_c")
    nc.vector.memset(eps_c[:], EPS)
    shift_c = g_const.tile([128, 1], FP, tag="shift_c", name="shift_c")
    nc.vector.memset(shift_c[:], -SHIFT)
    bias_sb = [g_const.tile([128, 9], FP, tag=f"bias{f}", name=f"bias{f}") for f in range(NP)]
    for f in range(NP):
        nc.sync.dma_start(bias_sb[f][:], d_bias[f * 128:(f + 1) * 128, :])
    mask_a = g_const.tile([128, 128], BF, tag="mask_a", name="mask_a")
    nc.sync.dma_start(mask_a[:], d_masks[0:128, :])
    mask_b = g_const.tile([128, 128], BF, tag="mask_b", name="mask_b")
    nc.sync.dma_start(mask_b[:], d_masks[128:256, :])
    bv_row = g_const.tile([1, C], FP, tag="bv_row", name="bv_row")
    nc.sync.dma_start(bv_row[:], d_bvrow[:])
    bv_rb = g_const.tile([1, C], BF, tag="bv_rb", name="bv_rb")
    nc.vector.tensor_copy(bv_rb[:], bv_row[:])
    bv_bc = g_const.tile([128, C], BF, tag="bv_bc", name="bv_bc")
    nc.gpsimd.partition_broadcast(bv_bc[:], bv_rb[:])

    # ---- weights for attention part ----
    w_sb = [g_w.tile([128, 4 * C], BF, tag=f"w{c}", name=f"w{c}") for c in range(NP)]
    for c in range(NP):
        nc.sync.dma_start(w_sb[c][:], d_wqkvp[c * 128:(c + 1) * 128, :])

    # ---- persistent activation storage ----
    KT = [g_kqv.tile([128, T], BF, tag=f"KT{i}", name=f"KT{i}") for i in range(NP)]
    QT = [g_kqv.tile([128, TM], BF, tag=f"QT{i}", name=f"QT{i}") for i in range(NP)]
    Vsb = [g_kqv.tile([128, H * 65], BF, tag=f"V{t}", name=f"V{t}") for t in range(NT)]

    # ================= LN1 + QKV, pipelined per 512-token group =============
    ln_es = ExitStack()
    g_roll = ln_es.enter_context(tc.tile_pool(name="lnroll", bufs=2))
    g_bc = ln_es.enter_context(tc.tile_pool(name="lnbc", bufs=2))
    g_small = ln_es.enter_context(tc.tile_pool(name="lnsmall", bufs=1))
    sps = ln_es.enter_context(tc.tile_pool(name="statps", bufs=2, space="PSUM"))
    gps = ln_es.enter_context(tc.tile_pool(name="gemmps", bufs=3, space="PSUM"))

    def emit_ln_group(g):
        """LN1 stats + h/hm for token group g (cols g*512..(g+1)*512 of T,
        owned cols g*256..(g+1)*256 of TM). Returns per-group h/hm tiles."""
        csl = slice(g * 512, (g + 1) * 512)
        msl = slice(g * 256, (g + 1) * 256)
        stats = sps.tile([33, 512], FP, tag="stf", name="stf")
        mstats = sps.tile([33, 256], FP, tag="stm", name="stm")
        xbs, xbms = [], []
        for ci in range(NP):
            xt = g_roll.tile([128, 512], FP, tag="xr", name="xr")
            nc.sync.dma_start(xt[:], d_xT[ci * 128:(ci + 1) * 128, csl])
            xb = g_roll.tile([128, 512], BF, tag="xb", name="xb", bufs=8)
            nc.vector.tensor_copy(xb[:], xt[:])
            sq = g_roll.tile([128, 512], BF, tag="sq", name="sq")
            nc.vector.tensor_mul(sq[:], xb[:], xb[:])
            nc.tensor.matmul(stats[0:1, :], ones_bf[:], xb[:],
                             start=(ci == 0), stop=(ci == NP - 1),
                             skip_group_check=True)
            nc.tensor.matmul(stats[32:33, :], ones_bf[:], sq[:],
                             start=(ci == 0), stop=(ci == NP - 1),
                             skip_group_check=True)
            xbs.append(xb)
            xtm = g_roll.tile([128, 256], FP, tag="xrm", name="xrm")
            nc.sync.dma_start(xtm[:], d_xTm[ci * 128:(ci + 1) * 128, msl])
            xbm = g_roll.tile([128, 256], BF, tag="xbm", name="xbm", bufs=8)
            nc.vector.tensor_copy(xbm[:], xtm[:])
            sqm = g_roll.tile([128, 256], BF, tag="sqm", name="sqm")
            nc.vector.tensor_mul(sqm[:], xbm[:], xbm[:])
            nc.tensor.matmul(mstats[0:1, :], ones_bf[:], xbm[:],
                             start=(ci == 0), stop=(ci == NP - 1),
                             skip_group_check=True)
            nc.tensor.matmul(mstats[32:33, :], ones_bf[:], sqm[:],
                             start=(ci == 0), stop=(ci == NP - 1),
                             skip_group_check=True)
            xbms.append(xbm)
        a_row = g_small.tile([1, 512], BF, tag="a_row", name="a_row")
        c_row = g_small.tile([1, 512], BF, tag="c_row", name="c_row")
        _ln_smalls(nc, g_small, stats, 512, eps_c, a_row, c_row)
        a_bc = g_bc.tile([128, 512], BF, tag="a_bc", name="a_bc")
        c_bc = g_bc.tile([128, 512], BF, tag="c_bc", name="c_bc")
        nc.gpsimd.partition_broadcast(a_bc[:], a_row[:])
        nc.gpsimd.partition_broadcast(c_bc[:], c_row[:])
        am_row = g_small.tile([1, 256], BF, tag="am_row", name="am_row")
        cm_row = g_small.tile([1, 256], BF, tag="cm_row", name="cm_row")
        _ln_smalls(nc, g_small, mstats, 256, eps_c, am_row, cm_row)
        am_bc = g_bc.tile([128, 256], BF, tag="am_bc", name="am_bc")
        cm_bc = g_bc.tile([128, 256], BF, tag="cm_bc", name="cm_bc")
        nc.gpsimd.partition_broadcast(am_bc[:], am_row[:])
        nc.gpsimd.partition_broadcast(cm_bc[:], cm_row[:])
        hg, hmg = [], []
        for ci in range(NP):
            tmp = g_roll.tile([128, 512], BF, tag="h_tmp", name="h_tmp")
            nc.vector.tensor_mul(tmp[:], xbs[ci][:], a_bc[:])
            hgc = g_roll.tile([128, 512], BF, tag=f"h{ci}", name="hgc")
            nc.vector.tensor_add(hgc[:], tmp[:], c_bc[:])
            hg.append(hgc)
            tmpm = g_roll.tile([128, 256], BF, tag="hm_tmp", name="hm_tmp")
            nc.vector.tensor_mul(tmpm[:], xbms[ci][:], am_bc[:])
            hmgc = g_roll.tile([128, 256], BF, tag=f"hm{ci}", name="hmgc")
            nc.vector.tensor_add(hmgc[:], tmpm[:], cm_bc[:])
            hmg.append(hmgc)
        if "h" in dbg:
            for ci in range(NP):
                nc.sync.dma_start(dbg["h"][ci * 128:(ci + 1) * 128, csl], hg[ci][:])
        if "hm" in dbg:
            for ci in range(NP):
                nc.sync.dma_start(dbg["hm"][ci * 128:(ci + 1) * 128, msl], hmg[ci][:])
        return hg, hmg

    def emit_qkv_group(g, hg, hmg):
        """K/Q/V GEMMs for token group g."""
        csl = slice(g * 512, (g + 1) * 512)
        msl = slice(g * 256, (g + 1) * 256)
        # K^T [C, T]: lhsT = wk tile, rhs = h
        for f in range(NP):
            ps = gps.tile([128, 512], FP, tag="ps", name="ps")
            for c in range(NP):
                nc.tensor.matmul(ps[:], w_sb[c][:, C + f * 128:C + (f + 1) * 128],
                                 hg[c][:], start=(c == 0), stop=(c == NP - 1))
            nc.scalar.activation(KT[f][:, csl], ps[:], AF.Identity,
                                 bias=bias_sb[f][:, 1:2])
        # Q^T [C, TM] from h_mine
        for f in range(NP):
            ps = gps.tile([128, 512], FP, tag="ps", name="ps")
            for c in range(NP):
                nc.tensor.matmul(ps[:, 0:256], w_sb[c][:, f * 128:(f + 1) * 128],
                                 hmg[c][:], start=(c == 0), stop=(c == NP - 1))
            nc.scalar.activation(QT[f][:, msl], ps[:, 0:256], AF.Identity,
                                 bias=bias_sb[f][:, 0:1])
        # V natural [T, C] (+ ones col per head): lhsT = h tile, rhs = wv
        for t in range(4):
            v3 = Vsb[4 * g + t][:].rearrange("p (h d) -> p h d", d=65)
            nc.vector.memset(v3[:, :, 64:65], 1.0)
            for fs in range(2):
                n = 512 if fs == 0 else 256
                nh = n // 64
                ps = gps.tile([128, 512], FP, tag="ps", name="ps")
                for c in range(NP):
                    nc.tensor.matmul(ps[:, 0:n], hg[c][:, t * 128:(t + 1) * 128],
                                     w_sb[c][:, 2 * C + fs * 512:2 * C + fs * 512 + n],
                                     start=(c == 0), stop=(c == NP - 1))
                nc.vector.scalar_tensor_tensor(
                    v3[:, fs * 8:fs * 8 + nh, 0:64],
                    ps[:, 0:n].rearrange("p (h d) -> p h d", d=64),
                    0.0,
                    bv_bc[:, fs * 512:fs * 512 + n].rearrange("p (h d) -> p h d", d=64),
                    AluOpType.add, AluOpType.add)

    cur = emit_ln_group(0)
    for g in range(NG):
        nxt = emit_ln_group(g + 1) if g + 1 < NG else None
        emit_qkv_group(g, *cur)
        cur = nxt

    for c in range(NP):
        if "KT" in dbg:
            nc.sync.dma_start(dbg["KT"][c * 128:(c + 1) * 128, :], KT[c][:])
        if "QT" in dbg:
            nc.sync.dma_start(dbg["QT"][c * 128:(c + 1) * 128, :], QT[c][:])
    if "V" in dbg:
        for t in range(NT):
            nc.sync.dma_start(dbg["V"][t * 128:(t + 1) * 128, :], Vsb[t][:])
    ln_es.close()

    attnT_es = ExitStack()
    g_attnT = attnT_es.enter_context(tc.tile_pool(name="attnT", bufs=1))
    attnT = [g_attnT.tile([128, TM], BF, tag=f"aT{i}", name=f"aT{i}") for i in range(NP)]

    # ================= attention (all 1024 owned query cols per head) =======
    att_es = ExitStack()
    g_wei = att_es.enter_context(tc.tile_pool(name="wei", bufs=3))
    g_asc = att_es.enter_context(tc.tile_pool(name="ascratch", bufs=1))
    g_rb = att_es.enter_context(tc.tile_pool(name="rbpool", bufs=2))
    ps_s_pool = att_es.enter_context(tc.tile_pool(name="sps2", bufs=2, space="PSUM"))
    ps_a_pool = att_es.enter_context(tc.tile_pool(name="aps", bufs=2, space="PSUM"))

    for hh in range(H):
        ht, hp = hh // 2, (hh % 2) * 64
        pa = ps_a_pool.tile([65, TM], FP, tag="pa", name="pa")
        for sb in range(NT):
            c0 = max(0, math.ceil((sb - 1) / 2)) * 128
            ps = ps_s_pool.tile([128, TM], FP, tag="ps", name="ps")
            if c0 < 512:
                nc.tensor.matmul(ps[:, c0:512],
                                 KT[ht][hp:hp + 64, sb * 128:(sb + 1) * 128],
                                 QT[ht][hp:hp + 64, c0:512],
                                 start=True, stop=True)
            nc.tensor.matmul(ps[:, max(c0, 512):TM],
                             KT[ht][hp:hp + 64, sb * 128:(sb + 1) * 128],
                             QT[ht][hp:hp + 64, max(c0, 512):TM],
                             start=True, stop=True)
            wei = g_wei.tile([128, TM], BF, tag="wei", name="wei")
            nc.scalar.activation(wei[:, c0:TM], ps[:, c0:TM], AF.Exp,
                                 bias=shift_c[:])
            if sb % 2 == 0:
                jd = sb // 2
                nc.vector.tensor_mul(wei[:, jd * 128:(jd + 1) * 128],
                                     wei[:, jd * 128:(jd + 1) * 128],
                                     mask_a[:])
            else:
                jd = (sb - 1) // 2
                nc.vector.tensor_mul(wei[:, jd * 128:(jd + 1) * 128],
                                     wei[:, jd * 128:(jd + 1) * 128],
                                     mask_b[:])
            if c0 < 512:
                nc.tensor.matmul(pa[:, c0:512], Vsb[sb][:, hh * 65:(hh + 1) * 65],
                                 wei[:, c0:512], start=(sb == 0),
                                 stop=(sb == 7), skip_group_check=True)
            nc.tensor.matmul(pa[:, max(c0, 512):TM],
                             Vsb[sb][:, hh * 65:(hh + 1) * 65],
                             wei[:, max(c0, 512):TM], start=(sb == 0),
                             stop=(sb == NT - 1), skip_group_check=True)
        sumrow = g_asc.tile([1, TM], FP, tag="sumrow", name="sumrow")
        nc.vector.tensor_copy(sumrow[:], pa[64:65, :])
        recip = g_asc.tile([1, TM], FP, tag="recip", name="recip")
        scr = g_asc.tile([1, TM], FP, tag="scr", name="scr")
        nc.vector.reciprocal_approx_accurate(recip[:], sumrow[:], scr[:])
        rb = g_rb.tile([64, TM], FP, tag="rb", name="rb")
        nc.gpsimd.partition_broadcast(rb[:], recip[:])
        nc.vector.tensor_mul(attnT[ht][hp:hp + 64, :], pa[0:64, :], rb[:])
    if "attnT" in dbg:
        for c in range(NP):
            nc.sync.dma_start(dbg["attnT"][c * 128:(c + 1) * 128, :], attnT[c][:])
    att_es.close()

    # ================= proj + residual =================
    proj_insts = []
    proj_es = ExitStack()
    gps2 = proj_es.enter_context(tc.tile_pool(name="pps", bufs=3, space="PSUM"))
    g_pr = proj_es.enter_context(tc.tile_pool(name="projroll", bufs=2))
    for f in range(NP):
        for g in range(TM // 512):
            ps = gps2.tile([128, 512], FP, tag="ps", name="ps")
            for c in range(NP):
                nc.tensor.matmul(ps[:], w_sb[c][:, 3 * C + f * 128:3 * C + (f + 1) * 128],
                                 attnT[c][:, g * 512:(g + 1) * 512],
                                 start=(c == 0), stop=(c == NP - 1))
            xr = g_pr.tile([128, 512], FP, tag="xr", name="xr")
            nc.sync.dma_start(xr[:], d_xTm[f * 128:(f + 1) * 128,
                                           g * 512:(g + 1) * 512])
            pi = nc.vector.scalar_tensor_tensor(
                xmid[f][:, g * 512:(g + 1) * 512], ps[:], bias_sb[f][:, 3:4],
                xr[:], AluOpType.add, AluOpType.add)
            proj_insts.append(pi)
    if "xmid" in dbg:
        for c in range(NP):
            nc.sync.dma_start(dbg["xmid"][c * 128:(c + 1) * 128, :], xmid[c][:])
    proj_es.close()
    attnT_es.close()
    kqv_stack.close()
    w_es.close()

    # ================= LN2 + MLP =================
    mlp_es = ExitStack()
    g_w1 = mlp_es.enter_context(tc.tile_pool(name="w1p", bufs=1))
    g_w2 = mlp_es.enter_context(tc.tile_pool(name="w2p", bufs=1))
    g_h2 = mlp_es.enter_context(tc.tile_pool(name="h2p", bufs=1))
    g_r = mlp_es.enter_context(tc.tile_pool(name="rp", bufs=2))
    g_roll2 = mlp_es.enter_context(tc.tile_pool(name="mlproll", bufs=2))
    g_bc2 = mlp_es.enter_context(tc.tile_pool(name="mlpbc", bufs=1))
    g_small2 = mlp_es.enter_context(tc.tile_pool(name="mlpsmall", bufs=1))
    sps2 = mlp_es.enter_context(tc.tile_pool(name="statps2", bufs=1, space="PSUM"))
    gps3 = mlp_es.enter_context(tc.tile_pool(name="mps", bufs=3, space="PSUM"))

    w1_sb = [g_w1.tile([128, 4 * C], BF, tag=f"w1_{c}", name=f"w1_{c}") for c in range(NP)]
    w2_sb = [g_w2.tile([128, 4 * C], BF, tag=f"w2_{c}", name=f"w2_{c}") for c in range(NP)]
    for c in range(NP):
        i1 = nc.sync.dma_start(w1_sb[c][:], d_w1[c * 128:(c + 1) * 128, :])
        i2 = nc.sync.dma_start(w2_sb[c][:], d_w2[c * 128:(c + 1) * 128, :])
        for pi in proj_insts:
            add_dep_helper(i1.ins, pi.ins, reason="mlp w-load after proj")
            add_dep_helper(i2.ins, pi.ins, reason="mlp w-load after proj")

    # LN2 stats from xmid (already in SBUF)
    ntg = TM // 512
    stats = [sps2.tile([33, 512], FP, tag=f"st2{g}", name=f"st2{g}") for g in range(ntg)]
    for ci in range(NP):
        xb = g_roll2.tile([128, TM], BF, tag="xb2", name="xb2")
        nc.vector.tensor_copy(xb[:], xmid[ci][:])
        sq = g_roll2.tile([128, TM], BF, tag="sq2", name="sq2")
        nc.vector.tensor_mul(sq[:], xb[:], xb[:])
        for g in range(ntg):
            nc.tensor.matmul(stats[g][0:1, :], ones_bf[:],
                             xb[:, g * 512:(g + 1) * 512],
                             start=(ci == 0), stop=(ci == NP - 1),
                             skip_group_check=True)
            nc.tensor.matmul(stats[g][32:33, :], ones_bf[:],
                             sq[:, g * 512:(g + 1) * 512],
                             start=(ci == 0), stop=(ci == NP - 1),
                             skip_group_check=True)
    a_bc2 = g_bc2.tile([128, TM], BF, tag="a2bc", name="a2bc")
    c_bc2 = g_bc2.tile([128, TM], BF, tag="c2bc", name="c2bc")
    for g in range(ntg):
        sl = slice(g * 512, (g + 1) * 512)
        a_row2 = g_small2.tile([1, 512], BF, tag="a_row2", name="a_row2")
        c_row2 = g_small2.tile([1, 512], BF, tag="c_row2", name="c_row2")
        _ln_smalls(nc, g_small2, stats[g], 512, eps_c, a_row2, c_row2)
        nc.gpsimd.partition_broadcast(a_bc2[:, sl], a_row2[:])
        nc.gpsimd.partition_broadcast(c_bc2[:, sl], c_row2[:])

    h2 = [g_h2.tile([128, TM], BF, tag=f"h2_{c}", name=f"h2_{c}") for c in range(NP)]
    for c in range(NP):
        tmp = g_roll2.tile([128, TM], FP, tag="h2tmp", name="h2tmp")
        nc.vector.tensor_mul(tmp[:], xmid[c][:], a_bc2[:])
        nc.vector.tensor_add(h2[c][:], tmp[:], c_bc2[:])

    for g in range(TM // 512):
        r_tiles = []
        for m in range(24):
            ps = gps3.tile([128, 512], FP, tag="ps", name="ps")
            for c in range(NP):
                nc.tensor.matmul(ps[:], w1_sb[c][:, m * 128:(m + 1) * 128],
                                 h2[c][:, g * 512:(g + 1) * 512],
                                 start=(c == 0), stop=(c == NP - 1))
            r = g_r.tile([128, 512], BF, tag=f"r{m}", name=f"r{m}")
            nc.vector.tensor_scalar(r[:], ps[:], bias_sb[m % 6][:, 5 + m // 6:6 + m // 6],
                                    0.0, AluOpType.add, AluOpType.max)
            r_tiles.append(r)
        for f in range(NP):
            ps = gps3.tile([128, 512], FP, tag="ps", name="ps")
            for m in range(24):
                nc.tensor.matmul(ps[:], w2_sb[m // 4][:, (m % 4) * 768 + f * 128:
                                                      (m % 4) * 768 + (f + 1) * 128],
                                 r_tiles[m][:],
                                 start=(m == 0), stop=(m == 23))
            ot = g_roll2.tile([128, 512], FP, tag="ot", name="ot")
            nc.vector.scalar_tensor_tensor(ot[:], ps[:], bias_sb[f][:, 4:5],
                                           xmid[f][:, g * 512:(g + 1) * 512],
                                           AluOpType.add, AluOpType.add)
            nc.sync.dma_start(d_out[f * 128:(f + 1) * 128, g * 512:(g + 1) * 512],
                              ot[:])
    mlp_es.close()
    es.close()


# ---------------------------------------------------------------------------
# host side
# ---------------------------------------------------------------------------

def _mycols(half):
    blocks = np.arange(8) * 2 + half
    return (blocks[:, None] * 128 + np.arange(128)[None, :]).reshape(-1)


def _prep_inputs(x, wq, bq, wk, bk, wv, bv, w_proj, b_proj, w1, b1, w2, b2,
                 g1, beta1, g2, beta2):
    x = np.asarray(x, np.float32)
    wq_f = np.ascontiguousarray(np.transpose(np.asarray(wq, np.float32), (1, 0, 2)).reshape(C, C))
    wk_f = np.ascontiguousarray(np.transpose(np.asarray(wk, np.float32), (1, 0, 2)).reshape(C, C))
    wv_f = np.ascontiguousarray(np.transpose(np.asarray(wv, np.float32), (1, 0, 2)).reshape(C, C))
    g1 = np.asarray(g1, np.float32); beta1 = np.asarray(beta1, np.float32)
    g2 = np.asarray(g2, np.float32); beta2 = np.asarray(beta2, np.float32)
    w1 = np.asarray(w1, np.float32); w2 = np.asarray(w2, np.float32)
    w_proj = np.asarray(w_proj, np.float32)

    wq_g = g1[:, None] * wq_f
    wk_g = g1[:, None] * wk_f
    wv_g = g1[:, None] * wv_f
    bq_f = beta1 @ wq_f + np.asarray(bq, np.float32).reshape(-1)
    bk_f = beta1 @ wk_f + np.asarray(bk, np.float32).reshape(-1)
    bv_f = beta1 @ wv_f + np.asarray(bv, np.float32).reshape(-1)
    w1_g = g2[:, None] * w1
    b1_f = beta2 @ w1 + np.asarray(b1, np.float32)

    wqkvp = np.concatenate([wq_g, wk_g, wv_g, w_proj], axis=1).astype(bf16)
    w1p = w1_g.astype(bf16)
    w2p = np.ascontiguousarray(
        w2.reshape(6, 4, 128, C).transpose(0, 2, 1, 3).reshape(C, 4 * C)).astype(bf16)

    biasp = np.zeros((C, 9), np.float32)
    biasp[:, 0] = bq_f
    biasp[:, 1] = bk_f
    biasp[:, 2] = bv_f
    biasp[:, 3] = np.asarray(b_proj, np.float32)
    biasp[:, 4] = np.asarray(b2, np.float32)
    biasp[:, 5:9] = b1_f.reshape(4, C).T
    bvrow = bv_f.reshape(1, C).astype(np.float32)

    tri = np.tril(np.ones((128, 128), np.float32)).T  # [s, q]: 1 iff s <= q
    in_maps = []
    for core in range(8):
        b, half = core // 2, core % 2
        xT = np.ascontiguousarray(x[b].T)
        xTm = np.ascontiguousarray(xT[:, _mycols(half)])
        masks = np.zeros((256, 128), np.float32)
        if half == 0:
            masks[0:128] = tri
            masks[128:256] = 0.0
        else:
            masks[0:128] = 1.0
            masks[128:256] = tri
        in_maps.append({
            "xT": xT, "xTm": xTm,
            "wqkvp": wqkvp, "w1p": w1p, "w2p": w2p,
            "biasp": biasp, "bvrow": bvrow,
            "masks": masks.astype(bf16),
        })
    return in_maps


def _assemble(results, dtype):
    out = np.empty((B, T, C), dtype)
    for core in range(8):
        b, half = core // 2, core % 2
        out[b, _mycols(half), :] = results[core]["outT"].T
    return out


def kernel(**inputs):
    in_maps = _prep_inputs(**inputs)
    if "nc" not in _cache:
        _cache["nc"] = _build()
    res = bass_utils.run_bass_kernel_spmd(_cache["nc"], in_maps,
                                          core_ids=list(range(8)))
    return _assemble(res.results, np.asarray(inputs["x"]).dtype)


# revision 30
# speedup vs baseline: 1.4418x; 1.0853x over previous
"""Transformer block (LN -> 12-head causal attention -> residual -> LN -> MLP
-> residual) for B=4, T=2048, C=768 on 8 trn2 NeuronCores.

Sharding: core = (batch, token-half). Each core handles one batch's K/V in
full and produces the final output for half the tokens (even or odd 128-token
blocks, which balances the causal-attention triangle). No collectives; all
per-core structural differences are carried in input *data* (host-gathered
xTm, causal-boundary mask tiles) so a single SPMD program runs on all 8
cores.

On-chip layout is feature-major ("transposed", [C, T]). LN statistics are
computed with ones-vector matmuls on the tensor engine and the LN1+QKV
pipeline is interleaved per 512-token group so the PE never starves.
Attention processes all 1024 owned query columns per head in one pass
(1024-wide exp tiles halve the scalar-engine instruction count); softmax
row-sums ride a ones column appended to V and are inverted with the fast
Newton-Raphson reciprocal.
"""

import math
import os
import sys

for _p in ("/opt/trn_rl_repo", "/root/.axon_site/_ro/trn_rl_repo"):
    if os.path.isdir(_p) and _p not in sys.path:
        sys.path.append(_p)

import numpy as np
import ml_dtypes

import concourse.bacc as bacc
import concourse.tile as tile
import concourse.mybir as mybir
from concourse import bass_utils
from concourse.alu_op_type import AluOpType
from concourse.tile_rust import add_dep_helper

BF = mybir.dt.bfloat16
FP = mybir.dt.float32
F8 = mybir.dt.float8e4
DR = mybir.MatmulPerfMode.DoubleRow
AF = mybir.ActivationFunctionType

B, T, C, H, HD = 4, 2048, 768, 12, 64
EPS = 1e-5
SHIFT = 40.0  # constant softmax shift: exp(s - SHIFT); exact softmax
NP = C // 128  # 6 feature partition-tiles
NT = T // 128  # 16 token blocks
TM = T // 2    # 1024 tokens owned per core
NG = 4         # 512-token groups
bf16 = ml_dtypes.bfloat16
fp8 = ml_dtypes.float8_e4m3

_cache = {}


def _build(debug=False):
    nc = bacc.Bacc("TRN2", target_bir_lowering=False, debug=False)
    d_xT = nc.dram_tensor("xT", [C, T], FP, kind="ExternalInput").ap()
    d_xTm = nc.dram_tensor("xTm", [C, TM], FP, kind="ExternalInput").ap()
    d_wqkvp = nc.dram_tensor("wqkvp", [C, 4 * C], BF, kind="ExternalInput").ap()
    d_w1 = nc.dram_tensor("w1p", [C // 2, 2 * 4 * C], F8, kind="ExternalInput").ap()
    d_w2 = nc.dram_tensor("w2p", [12 * 128, 2 * C], F8, kind="ExternalInput").ap()
    d_bias = nc.dram_tensor("biasp", [C, 9], FP, kind="ExternalInput").ap()
    d_bvrow = nc.dram_tensor("bvrow", [1, C], FP, kind="ExternalInput").ap()
    d_masks = nc.dram_tensor("masks", [256, 128], BF, kind="ExternalInput").ap()
    d_out = nc.dram_tensor("outT", [C, TM], FP, kind="ExternalOutput").ap()
    if debug is True:
        debug = ["h", "hm", "KT", "QT", "V", "attnT", "xmid"]
    debug = debug or []
    dbg = {}
    if "h" in debug:
        dbg["h"] = nc.dram_tensor("dbg_h", [C, T], BF, kind="ExternalOutput").ap()
    if "hm" in debug:
        dbg["hm"] = nc.dram_tensor("dbg_hm", [C, TM], BF, kind="ExternalOutput").ap()
    if "KT" in debug:
        dbg["KT"] = nc.dram_tensor("dbg_KT", [C, T], BF, kind="ExternalOutput").ap()
    if "QT" in debug:
        dbg["QT"] = nc.dram_tensor("dbg_QT", [C, TM], BF, kind="ExternalOutput").ap()
    if "V" in debug:
        dbg["V"] = nc.dram_tensor("dbg_V", [T, H * 65], BF, kind="ExternalOutput").ap()
    if "attnT" in debug:
        dbg["attnT"] = nc.dram_tensor("dbg_attnT", [C, TM], BF, kind="ExternalOutput").ap()
    if "xmid" in debug:
        dbg["xmid"] = nc.dram_tensor("dbg_xmid", [C, TM], FP, kind="ExternalOutput").ap()

    with tile.TileContext(nc) as tc:
        _body(nc, tc, d_xT, d_xTm, d_wqkvp, d_w1, d_w2, d_bias, d_bvrow,
              d_masks, d_out, dbg)
    nc.compile()
    return nc


def _ln_smalls(nc, small, stats, ncols, eps_c, a_dst, c_dst):
    """From accumulated [33, ncols] stats (row 0 = sum, row 32 = sumsq),
    produce bf16 [1, ncols] rows a5b (1/std) and c5b (-mu/std)."""
    mu = small.tile([1, ncols], FP, tag=f"mu{ncols}", name="mu")
    nc.scalar.mul(mu[:], stats[0:1, :], 1.0 / C)
    m2 = small.tile([1, ncols], FP, tag=f"m2{ncols}", name="m2")
    nc.scalar.mul(m2[:], stats[32:33, :], 1.0 / C)
    var = small.tile([1, ncols], FP, tag=f"va{ncols}", name="va")
    nc.vector.tensor_mul(var[:], mu[:], mu[:])
    nc.vector.tensor_sub(var[:], m2[:], var[:])
    std = small.tile([1, ncols], FP, tag=f"sd{ncols}", name="sd")
    nc.scalar.activation(std[:], var[:], AF.Sqrt, bias=eps_c[0:1, 0:1])
    a5 = small.tile([1, ncols], FP, tag=f"a5{ncols}", name="a5")
    nc.vector.reciprocal_approx_accurate(a5[:], std[:], var[:])
    c5 = small.tile([1, ncols], FP, tag=f"c5{ncols}", name="c5")
    nc.vector.scalar_tensor_tensor(c5[:], mu[:], -1.0, a5[:],
                                   AluOpType.mult, AluOpType.mult)
    nc.vector.tensor_copy(a_dst[:], a5[:])
    nc.vector.tensor_copy(c_dst[:], c5[:])


def _body(nc, tc, d_xT, d_xTm, d_wqkvp, d_w1, d_w2, d_bias, d_bvrow,
          d_masks, d_out, dbg={}):
    from contextlib import ExitStack

    es = ExitStack()
    g_const = es.enter_context(tc.tile_pool(name="const", bufs=1))
    g_xmid = es.enter_context(tc.tile_pool(name="xmid", bufs=1))
    xmid = [g_xmid.tile([128, TM], FP, tag=f"xm{i}", name=f"xm{i}") for i in range(NP)]
    w_es = ExitStack()
    kqv_stack = ExitStack()
    g_w = w_es.enter_context(tc.tile_pool(name="wqkvp", bufs=1))
    g_kqv = kqv_stack.enter_context(tc.tile_pool(name="kqv", bufs=1))

    # ---- constants ----
    ones_bf = g_const.tile([128, 1], BF, tag="ones_bf", name="ones_bf")
    nc.vector.memset(ones_bf[:], 1.0)
    eps_c = g_const.tile([128, 1], FP, tag="eps_c", name="eps_c")
    nc.vector.memset(eps_c[:], EPS)
    shift_c = g_const.tile([128, 1], FP, tag="shift_c", name="shift_c")
    nc.vector.memset(shift_c[:], -SHIFT)
    bias_sb = [g_const.tile([128, 9], FP, tag=f"bias{f}", name=f"bias{f}") for f in range(NP)]
    for f in range(NP):
        nc.sync.dma_start(bias_sb[f][:], d_bias[f * 128:(f + 1) * 128, :])
    mask_a = g_const.tile([128, 128], BF, tag="mask_a", name="mask_a")
    nc.sync.dma_start(mask_a[:], d_masks[0:128, :])
    mask_b = g_const.tile([128, 128], BF, tag="mask_b", name="mask_b")
    nc.sync.dma_start(mask_b[:], d_masks[128:256, :])
    bv_row = g_const.tile([1, C], FP, tag="bv_row", name="bv_row")
    nc.sync.dma_start(bv_row[:], d_bvrow[:])
    bv_rb = g_const.tile([1, C], BF, tag="bv_rb", name="bv_rb")
    nc.vector.tensor_copy(bv_rb[:], bv_row[:])
    bv_bc = g_const.tile([128, C], BF, tag="bv_bc", name="bv_bc")
    nc.gpsimd.partition_broadcast(bv_bc[:], bv_rb[:])

    # ---- weights for attention part ----
    w_sb = [g_w.tile([128, 4 * C], BF, tag=f"w{c}", name=f"w{c}") for c in range(NP)]
    for c in range(NP):
        nc.sync.dma_start(w_sb[c][:], d_wqkvp[c * 128:(c + 1) * 128, :])

    # ---- persistent activation storage ----
    KT = [g_kqv.tile([128, T], BF, tag=f"KT{i}", name=f"KT{i}") for i in range(NP)]
    QT = [g_kqv.tile([128, TM], BF, tag=f"QT{i}", name=f"QT{i}") for i in range(NP)]
    Vsb = [g_kqv.tile([128, H * 65], BF, tag=f"V{t}", name=f"V{t}") for t in range(NT)]

    # ================= LN1 + QKV, pipelined per 512-token group =============
    ln_es = ExitStack()
    g_roll = ln_es.enter_context(tc.tile_pool(name="lnroll", bufs=2))
    g_bc = ln_es.enter_context(tc.tile_pool(name="lnbc", bufs=2))
    g_small = ln_es.enter_context(tc.tile_pool(name="lnsmall", bufs=1))
    sps = ln_es.enter_context(tc.tile_pool(name="statps", bufs=2, space="PSUM"))
    gps = ln_es.enter_context(tc.tile_pool(name="gemmps", bufs=3, space="PSUM"))

    def emit_ln_group(g):
        """LN1 stats + h/hm for token group g (cols g*512..(g+1)*512 of T,
        owned cols g*256..(g+1)*256 of TM). Returns per-group h/hm tiles."""
        csl = slice(g * 512, (g + 1) * 512)
        msl = slice(g * 256, (g + 1) * 256)
        stats = sps.tile([33, 512], FP, tag="stf", name="stf")
        mstats = sps.tile([33, 256], FP, tag="stm", name="stm")
        xbs, xbms = [], []
        for ci in range(NP):
            xt = g_roll.tile([128, 512], FP, tag="xr", name="xr")
            nc.sync.dma_start(xt[:], d_xT[ci * 128:(ci + 1) * 128, csl])
            xb = g_roll.tile([128, 512], BF, tag="xb", name="xb", bufs=8)
            nc.vector.tensor_copy(xb[:], xt[:])
            sq = g_roll.tile([128, 512], BF, tag="sq", name="sq")
            nc.vector.tensor_mul(sq[:], xb[:], xb[:])
            nc.tensor.matmul(stats[0:1, :], ones_bf[:], xb[:],
                             start=(ci == 0), stop=(ci == NP - 1),
                             skip_group_check=True)
            nc.tensor.matmul(stats[32:33, :], ones_bf[:], sq[:],
                             start=(ci == 0), stop=(ci == NP - 1),
                             skip_group_check=True)
            xbs.append(xb)
            xtm = g_roll.tile([128, 256], FP, tag="xrm", name="xrm")
            nc.sync.dma_start(xtm[:], d_xTm[ci * 128:(ci + 1) * 128, msl])
            xbm = g_roll.tile([128, 256], BF, tag="xbm", name="xbm", bufs=8)
            nc.vector.tensor_copy(xbm[:], xtm[:])
            sqm = g_roll.tile([128, 256], BF, tag="sqm", name="sqm")
            nc.vector.tensor_mul(sqm[:], xbm[:], xbm[:])
            nc.tensor.matmul(mstats[0:1, :], ones_bf[:], xbm[:],
                             start=(ci == 0), stop=(ci == NP - 1),
                             skip_group_check=True)
            nc.tensor.matmul(mstats[32:33, :], ones_bf[:], sqm[:],
                             start=(ci == 0), stop=(ci == NP - 1),
                             skip_group_check=True)
            xbms.append(xbm)
        a_row = g_small.tile([1, 512], BF, tag="a_row", name="a_row")
        c_row = g_small.tile([1, 512], BF, tag="c_row", name="c_row")
        _ln_smalls(nc, g_small, stats, 512, eps_c, a_row, c_row)
        a_bc = g_bc.tile([128, 512], BF, tag="a_bc", name="a_bc")
        c_bc = g_bc.tile([128, 512], BF, tag="c_bc", name="c_bc")
        nc.gpsimd.partition_broadcast(a_bc[:], a_row[:])
        nc.gpsimd.partition_broadcast(c_bc[:], c_row[:])
        am_row = g_small.tile([1, 256], BF, tag="am_row", name="am_row")
        cm_row = g_small.tile([1, 256], BF, tag="cm_row", name="cm_row")
        _ln_smalls(nc, g_small, mstats, 256, eps_c, am_row, cm_row)
        am_bc = g_bc.tile([128, 256], BF, tag="am_bc", name="am_bc")
        cm_bc = g_bc.tile([128, 256], BF, tag="cm_bc", name="cm_bc")
        nc.gpsimd.partition_broadcast(am_bc[:], am_row[:])
        nc.gpsimd.partition_broadcast(cm_bc[:], cm_row[:])
        hg, hmg = [], []
        for ci in range(NP):
            tmp = g_roll.tile([128, 512], BF, tag="h_tmp", name="h_tmp")
            nc.vector.tensor_mul(tmp[:], xbs[ci][:], a_bc[:])
            hgc = g_roll.tile([128, 512], BF, tag=f"h{ci}", name="hgc")
            nc.vector.tensor_add(hgc[:], tmp[:], c_bc[:])
            hg.append(hgc)
            tmpm = g_roll.tile([128, 256], BF, tag="hm_tmp", name="hm_tmp")
            nc.vector.tensor_mul(tmpm[:], xbms[ci][:], am_bc[:])
            hmgc = g_roll.tile([128, 256], BF, tag=f"hm{ci}", name="hmgc")
            nc.vector.tensor_add(hmgc[:], tmpm[:], cm_bc[:])
            hmg.append(hmgc)
        if "h" in dbg:
            for ci in range(NP):
                nc.sync.dma_start(dbg["h"][ci * 128:(ci + 1) * 128, csl], hg[ci][:])
        if "hm" in dbg:
            for ci in range(NP):
                nc.sync.dma_start(dbg["hm"][ci * 128:(ci + 1) * 128, msl], hmg[ci][:])
        return hg, hmg

    def emit_qkv_group(g, hg, hmg):
        """K/Q/V GEMMs for token group g."""
        csl = slice(g * 512, (g + 1) * 512)
        msl = slice(g * 256, (g + 1) * 256)
        # K^T [C, T]: lhsT = wk tile, rhs = h
        for f in range(NP):
            ps = gps.tile([128, 512], FP, tag="ps", name="ps")
            for c in range(NP):
                nc.tensor.matmul(ps[:], w_sb[c][:, C + f * 128:C + (f + 1) * 128],
                                 hg[c][:], start=(c == 0), stop=(c == NP - 1))
            nc.scalar.activation(KT[f][:, csl], ps[:], AF.Identity,
                                 bias=bias_sb[f][:, 1:2])
        # Q^T [C, TM] from h_mine
        for f in range(NP):
            ps = gps.tile([128, 512], FP, tag="ps", name="ps")
            for c in range(NP):
                nc.tensor.matmul(ps[:, 0:256], w_sb[c][:, f * 128:(f + 1) * 128],
                                 hmg[c][:], start=(c == 0), stop=(c == NP - 1))
            nc.scalar.activation(QT[f][:, msl], ps[:, 0:256], AF.Identity,
                                 bias=bias_sb[f][:, 0:1])
        # V natural [T, C] (+ ones col per head): lhsT = h tile, rhs = wv
        for t in range(4):
            v3 = Vsb[4 * g + t][:].rearrange("p (h d) -> p h d", d=65)
            nc.vector.memset(v3[:, :, 64:65], 1.0)
            for fs in range(2):
                n = 512 if fs == 0 else 256
                nh = n // 64
                ps = gps.tile([128, 512], FP, tag="ps", name="ps")
                for c in range(NP):
                    nc.tensor.matmul(ps[:, 0:n], hg[c][:, t * 128:(t + 1) * 128],
                                     w_sb[c][:, 2 * C + fs * 512:2 * C + fs * 512 + n],
                                     start=(c == 0), stop=(c == NP - 1))
                nc.vector.scalar_tensor_tensor(
                    v3[:, fs * 8:fs * 8 + nh, 0:64],
                    ps[:, 0:n].rearrange("p (h d) -> p h d", d=64),
                    0.0,
                    bv_bc[:, fs * 512:fs * 512 + n].rearrange("p (h d) -> p h d", d=64),
                    AluOpType.add, AluOpType.add)

    cur = emit_ln_group(0)
    for g in range(NG):
        nxt = emit_ln_group(g + 1) if g + 1 < NG else None
        emit_qkv_group(g, *cur)
        cur = nxt

    for c in range(NP):
        if "KT" in dbg:
            nc.sync.dma_start(dbg["KT"][c * 128:(c + 1) * 128, :], KT[c][:])
        if "QT" in dbg:
            nc.sync.dma_start(dbg["QT"][c * 128:(c + 1) * 128, :], QT[c][:])
    if "V" in dbg:
        for t in range(NT):
            nc.sync.dma_start(dbg["V"][t * 128:(t + 1) * 128, :], Vsb[t][:])
    ln_es.close()

    attnT_es = ExitStack()
    g_attnT = attnT_es.enter_context(tc.tile_pool(name="attnT", bufs=1))
    attnT = [g_attnT.tile([128, TM], BF, tag=f"aT{i}", name=f"aT{i}") for i in range(NP)]

    # ================= attention (all 1024 owned query cols per head) =======
    att_es = ExitStack()
    g_wei = att_es.enter_context(tc.tile_pool(name="wei", bufs=3))
    g_asc = att_es.enter_context(tc.tile_pool(name="ascratch", bufs=1))
    g_rb = att_es.enter_context(tc.tile_pool(name="rbpool", bufs=2))
    ps_s_pool = att_es.enter_context(tc.tile_pool(name="sps2", bufs=2, space="PSUM"))
    ps_a_pool = att_es.enter_context(tc.tile_pool(name="aps", bufs=2, space="PSUM"))

    for hh in range(H):
        ht, hp = hh // 2, (hh % 2) * 64
        pa = ps_a_pool.tile([65, TM], FP, tag="pa", name="pa")
        for sb in range(NT):
            c0 = max(0, math.ceil((sb - 1) / 2)) * 128
            ps = ps_s_pool.tile([128, TM], FP, tag="ps", name="ps")
            if c0 < 512:
                nc.tensor.matmul(ps[:, c0:512],
                                 KT[ht][hp:hp + 64, sb * 128:(sb + 1) * 128],
                                 QT[ht][hp:hp + 64, c0:512],
                                 start=True, stop=True)
            nc.tensor.matmul(ps[:, max(c0, 512):TM],
                             KT[ht][hp:hp + 64, sb * 128:(sb + 1) * 128],
                             QT[ht][hp:hp + 64, max(c0, 512):TM],
                             start=True, stop=True)
            wei = g_wei.tile([128, TM], BF, tag="wei", name="wei")
            nc.scalar.activation(wei[:, c0:TM], ps[:, c0:TM], AF.Exp,
                                 bias=shift_c[:])
            if sb % 2 == 0:
                jd = sb // 2
                nc.vector.tensor_mul(wei[:, jd * 128:(jd + 1) * 128],
                                     wei[:, jd * 128:(jd + 1) * 128],
                                     mask_a[:])
            else:
                jd = (sb - 1) // 2
                nc.vector.tensor_mul(wei[:, jd * 128:(jd + 1) * 128],
                                     wei[:, jd * 128:(jd + 1) * 128],
                                     mask_b[:])
            if c0 < 512:
                nc.tensor.matmul(pa[:, c0:512], Vsb[sb][:, hh * 65:(hh + 1) * 65],
                                 wei[:, c0:512], start=(sb == 0),
                                 stop=(sb == 7), skip_group_check=True)
            nc.tensor.matmul(pa[:, max(c0, 512):TM],
                             Vsb[sb][:, hh * 65:(hh + 1) * 65],
                             wei[:, max(c0, 512):TM], start=(sb == 0),
                             stop=(sb == NT - 1), skip_group_check=True)
        sumrow = g_asc.tile([1, TM], FP, tag="sumrow", name="sumrow")
        nc.vector.tensor_copy(sumrow[:], pa[64:65, :])
        recip = g_asc.tile([1, TM], FP, tag="recip", name="recip")
        scr = g_asc.tile([1, TM], FP, tag="scr", name="scr")
        nc.vector.reciprocal_approx_accurate(recip[:], sumrow[:], scr[:])
        rb = g_rb.tile([64, TM], FP, tag="rb", name="rb")
        nc.gpsimd.partition_broadcast(rb[:], recip[:])
        nc.vector.tensor_mul(attnT[ht][hp:hp + 64, :], pa[0:64, :], rb[:])
    if "attnT" in dbg:
        for c in range(NP):
            nc.sync.dma_start(dbg["attnT"][c * 128:(c + 1) * 128, :], attnT[c][:])
    att_es.close()

    # ================= proj + residual =================
    proj_insts = []
    proj_es = ExitStack()
    gps2 = proj_es.enter_context(tc.tile_pool(name="pps", bufs=3, space="PSUM"))
    g_pr = proj_es.enter_context(tc.tile_pool(name="projroll", bufs=2))
    for f in range(NP):
        for g in range(TM // 512):
            ps = gps2.tile([128, 512], FP, tag="ps", name="ps")
            for c in range(NP):
                nc.tensor.matmul(ps[:], w_sb[c][:, 3 * C + f * 128:3 * C + (f + 1) * 128],
                                 attnT[c][:, g * 512:(g + 1) * 512],
                                 start=(c == 0), stop=(c == NP - 1))
            xr = g_pr.tile([128, 512], FP, tag="xr", name="xr")
            nc.sync.dma_start(xr[:], d_xTm[f * 128:(f + 1) * 128,
                                           g * 512:(g + 1) * 512])
            pi = nc.vector.scalar_tensor_tensor(
                xmid[f][:, g * 512:(g + 1) * 512], ps[:], bias_sb[f][:, 3:4],
                xr[:], AluOpType.add, AluOpType.add)
            proj_insts.append(pi)
    if "xmid" in dbg:
        for c in range(NP):
            nc.sync.dma_start(dbg["xmid"][c * 128:(c + 1) * 128, :], xmid[c][:])
    proj_es.close()
    attnT_es.close()
    kqv_stack.close()
    w_es.close()

    # ================= LN2 + MLP =================
    mlp_es = ExitStack()
    g_w1 = mlp_es.enter_context(tc.tile_pool(name="w1p", bufs=1))
    g_w2 = mlp_es.enter_context(tc.tile_pool(name="w2p", bufs=1))
    g_h2 = mlp_es.enter_context(tc.tile_pool(name="h2p", bufs=1))
    g_r = mlp_es.enter_context(tc.tile_pool(name="rp", bufs=2))
    g_roll2 = mlp_es.enter_context(tc.tile_pool(name="mlproll", bufs=2))
    g_bc2 = mlp_es.enter_context(tc.tile_pool(name="mlpbc", bufs=1))
    g_small2 = mlp_es.enter_context(tc.tile_pool(name="mlpsmall", bufs=1))
    sps2 = mlp_es.enter_context(tc.tile_pool(name="statps2", bufs=1, space="PSUM"))
    gps3 = mlp_es.enter_context(tc.tile_pool(name="mps", bufs=3, space="PSUM"))

    w1_sb = [g_w1.tile([128, 2 * 4 * C], F8, tag=f"w1_{c}", name=f"w1_{c}") for c in range(3)]
    w2_sb = [g_w2.tile([128, 2 * C], F8, tag=f"w2_{c}", name=f"w2_{c}") for c in range(12)]
    for c in range(3):
        i1 = nc.sync.dma_start(w1_sb[c][:], d_w1[c * 128:(c + 1) * 128, :])
        for pi in proj_insts:
            add_dep_helper(i1.ins, pi.ins, reason="mlp w-load after proj")
    for c in range(12):
        i2 = nc.sync.dma_start(w2_sb[c][:], d_w2[c * 128:(c + 1) * 128, :])
        for pi in proj_insts:
            add_dep_helper(i2.ins, pi.ins, reason="mlp w-load after proj")

    # LN2 stats from xmid (already in SBUF)
    ntg = TM // 512
    stats = [sps2.tile([33, 512], FP, tag=f"st2{g}", name=f"st2{g}") for g in range(ntg)]
    for ci in range(NP):
        xb = g_roll2.tile([128, TM], BF, tag="xb2", name="xb2")
        nc.vector.tensor_copy(xb[:], xmid[ci][:])
        sq = g_roll2.tile([128, TM], BF, tag="sq2", name="sq2")
        nc.vector.tensor_mul(sq[:], xb[:], xb[:])
        for g in range(ntg):
            nc.tensor.matmul(stats[g][0:1, :], ones_bf[:],
                             xb[:, g * 512:(g + 1) * 512],
                             start=(ci == 0), stop=(ci == NP - 1),
                             skip_group_check=True)
            nc.tensor.matmul(stats[g][32:33, :], ones_bf[:],
                             sq[:, g * 512:(g + 1) * 512],
                             start=(ci == 0), stop=(ci == NP - 1),
                             skip_group_check=True)
    a_bc2 = g_bc2.tile([128, TM], BF, tag="a2bc", name="a2bc")
    c_bc2 = g_bc2.tile([128, TM], BF, tag="c2bc", name="c2bc")
    for g in range(ntg):
        sl = slice(g * 512, (g + 1) * 512)
        a_row2 = g_small2.tile([1, 512], BF, tag="a_row2", name="a_row2")
        c_row2 = g_small2.tile([1, 512], BF, tag="c_row2", name="c_row2")
        _ln_smalls(nc, g_small2, stats[g], 512, eps_c, a_row2, c_row2)
        nc.gpsimd.partition_broadcast(a_bc2[:, sl], a_row2[:])
        nc.gpsimd.partition_broadcast(c_bc2[:, sl], c_row2[:])

    h2 = [g_h2.tile([128, 2 * TM], F8, tag=f"h2_{c}", name=f"h2_{c}") for c in range(3)]
    for c in range(NP):
        tmp = g_roll2.tile([128, TM], FP, tag="h2tmp", name="h2tmp")
        nc.vector.tensor_mul(tmp[:], xmid[c][:], a_bc2[:])
        nc.vector.tensor_add(h2[c // 2][:, (c % 2) * TM:(c % 2 + 1) * TM],
                             tmp[:], c_bc2[:])

    for g in range(TM // 512):
        r_tiles = []
        for m in range(24):
            ps = gps3.tile([128, 512], FP, tag="ps", name="ps")
            for pk in range(3):
                w3 = w1_sb[pk][:].rearrange("p (two n) -> p two n", two=2)
                h3 = h2[pk][:].rearrange("p (two t) -> p two t", two=2)
                nc.tensor.matmul(ps[:], w3[:, :, m * 128:(m + 1) * 128],
                                 h3[:, :, g * 512:(g + 1) * 512],
                                 start=(pk == 0), stop=(pk == 2), perf_mode=DR)
            if m % 2 == 0:
                r = g_r.tile([128, 2 * 512], F8, tag=f"r{m // 2}", name=f"r{m // 2}")
                r_tiles.append(r)
            nc.vector.tensor_scalar(r_tiles[m // 2][:, (m % 2) * 512:(m % 2 + 1) * 512],
                                    ps[:], bias_sb[m % 6][:, 5 + m // 6:6 + m // 6],
                                    0.0, AluOpType.add, AluOpType.max)
        for f in range(NP):
            ps = gps3.tile([128, 512], FP, tag="ps", name="ps")
            for pm in range(12):
                w3 = w2_sb[pm][:].rearrange("p (two n) -> p two n", two=2)
                r3 = r_tiles[pm][:].rearrange("p (two t) -> p two t", two=2)
                nc.tensor.matmul(ps[:], w3[:, :, f * 128:(f + 1) * 128], r3[:],
                                 start=(pm == 0), stop=(pm == 11), perf_mode=DR)
            ot = g_roll2.tile([128, 512], FP, tag="ot", name="ot")
            nc.vector.scalar_tensor_tensor(ot[:], ps[:], bias_sb[f][:, 4:5],
                                           xmid[f][:, g * 512:(g + 1) * 512],
                                           AluOpType.add, AluOpType.add)
            nc.sync.dma_start(d_out[f * 128:(f + 1) * 128, g * 512:(g + 1) * 512],
                              ot[:])
    mlp_es.close()
    es.close()


# ---------------------------------------------------------------------------
# host side
# ---------------------------------------------------------------------------

def _mycols(half):
    blocks = np.arange(8) * 2 + half
    return (blocks[:, None] * 128 + np.arange(128)[None, :]).reshape(-1)


def _prep_inputs(x, wq, bq, wk, bk, wv, bv, w_proj, b_proj, w1, b1, w2, b2,
                 g1, beta1, g2, beta2):
    x = np.asarray(x, np.float32)
    wq_f = np.ascontiguousarray(np.transpose(np.asarray(wq, np.float32), (1, 0, 2)).reshape(C, C))
    wk_f = np.ascontiguousarray(np.transpose(np.asarray(wk, np.float32), (1, 0, 2)).reshape(C, C))
    wv_f = np.ascontiguousarray(np.transpose(np.asarray(wv, np.float32), (1, 0, 2)).reshape(C, C))
    g1 = np.asarray(g1, np.float32); beta1 = np.asarray(beta1, np.float32)
    g2 = np.asarray(g2, np.float32); beta2 = np.asarray(beta2, np.float32)
    w1 = np.asarray(w1, np.float32); w2 = np.asarray(w2, np.float32)
    w_proj = np.asarray(w_proj, np.float32)

    wq_g = g1[:, None] * wq_f
    wk_g = g1[:, None] * wk_f
    wv_g = g1[:, None] * wv_f
    bq_f = beta1 @ wq_f + np.asarray(bq, np.float32).reshape(-1)
    bk_f = beta1 @ wk_f + np.asarray(bk, np.float32).reshape(-1)
    bv_f = beta1 @ wv_f + np.asarray(bv, np.float32).reshape(-1)
    w1_g = g2[:, None] * w1
    b1_f = beta2 @ w1 + np.asarray(b1, np.float32)

    wqkvp = np.concatenate([wq_g, wk_g, wv_g, w_proj], axis=1).astype(bf16)
    w1p = np.ascontiguousarray(
        w1_g.reshape(3, 2, 128, 4 * C).transpose(0, 2, 1, 3).reshape(384, 2 * 4 * C))
    w1p = np.clip(w1p, -240, 240).astype(fp8)
    w2p = np.ascontiguousarray(
        w2.reshape(12, 2, 128, C).transpose(0, 2, 1, 3).reshape(12 * 128, 2 * C))
    w2p = np.clip(w2p, -240, 240).astype(fp8)

    biasp = np.zeros((C, 9), np.float32)
    biasp[:, 0] = bq_f
    biasp[:, 1] = bk_f
    biasp[:, 2] = bv_f
    biasp[:, 3] = np.asarray(b_proj, np.float32)
    biasp[:, 4] = np.asarray(b2, np.float32)
    biasp[:, 5:9] = b1_f.reshape(4, C).T
    bvrow = bv_f.reshape(1, C).astype(np.float32)

    tri = np.tril(np.ones((128, 128), np.float32)).T  # [s, q]: 1 iff s <= q
    in_maps = []
    for core in range(8):
        b, half = core // 2, core % 2
        xT = np.ascontiguousarray(x[b].T)
        xTm = np.ascontiguousarray(xT[:, _mycols(half)])
        masks = np.zeros((256, 128), np.float32)
        if half == 0:
            masks[0:128] = tri
            masks[128:256] = 0.0
        else:
            masks[0:128] = 1.0
            masks[128:256] = tri
        in_maps.append({
            "xT": xT, "xTm": xTm,
            "wqkvp": wqkvp, "w1p": w1p, "w2p": w2p,
            "biasp": biasp, "bvrow": bvrow,
            "masks": masks.astype(bf16),
        })
    return in_maps


def _assemble(results, dtype):
    out = np.empty((B, T, C), dtype)
    for core in range(8):
        b, half = core // 2, core % 2
        out[b, _mycols(half), :] = results[core]["outT"].T
    return out


def kernel(**inputs):
    in_maps = _prep_inputs(**inputs)
    if "nc" not in _cache:
        _cache["nc"] = _build()
    res = bass_utils.run_bass_kernel_spmd(_cache["nc"], in_maps,
                                          core_ids=list(range(8)))
    return _assemble(res.results, np.asarray(inputs["x"]).dtype)
